# revision 1
# baseline (speedup 1.0000x reference)
"""TAGConv GNN classifier on 8 Trainium2 NeuronCores.

Sharding: nodes split into 8 contiguous slices (6250/core, padded to 6272);
edges live on the core that owns their dst. Each hop: every core gathers
src rows from a replicated norm-prescaled node table in HBM (dma_gather,
int16 indices -> split-table trick), segment-sums them into its dst slice
with one-hot matmuls on TensorE (PSUM accumulation), rescales by norm, and
all-gathers its slice of the next table. Readout partial sums per graph are
all-reduced, then every core computes the (identical) logits.
"""
import os

import numpy as np

import concourse.bass as bass
import concourse.bacc as bacc
import concourse.mybir as mybir
import concourse.tile as tile
from concourse import bass_utils

N, E, G = 50000, 800000, 128
F = 128                      # IN_DIM == HID
CLASSES = 10
HOPS, HLAYERS = 2, 2         # 3 TAGConv layers total
NCORES = 8


def configure(n, e):
    """Derived sizes; module-level so debug harnesses can shrink the problem."""
    global N, E, PER, GRP, NPAD, NT, HALF
    N, E = n, e
    PER = N // NCORES            # real nodes per core
    GRP = (PER + 127) // 128     # dst groups of 128 per core
    NPAD = GRP * 128             # padded nodes per core
    NT = NCORES * NPAD           # padded total
    HALF = NT // 2               # int16-safe split of the node table


configure(N, E)

FP = mybir.dt.float32
I16 = mybir.dt.int16


def _prep_edges(src, dst):
    """Per-core gather-index + one-hot-slot tables, SPMD-uniform shapes."""
    src = src.astype(np.int64)
    dst = dst.astype(np.int64)
    core = dst // PER
    local = dst - core * PER
    grp = local // 128
    slot = local % 128
    ps = (src // PER) * NPAD + (src % PER)          # padded global src id
    half = (ps >= HALF).astype(np.int64)
    idxv = ps - half * HALF                          # int16-safe index

    key = (core * GRP + grp) * 2 + half
    order = np.argsort(key, kind="stable")
    cnt = np.bincount(key, minlength=NCORES * GRP * 2).reshape(NCORES, GRP, 2)
    CA = np.maximum(1, -(-cnt[:, :, 0].max(axis=0) // 128)).astype(int)
    CB = np.maximum(1, -(-cnt[:, :, 1].max(axis=0) // 128)).astype(int)
    nch = CA + CB                                    # chunks per group
    choff = np.concatenate([[0], np.cumsum(nch)]).astype(int)
    NCH = int(choff[-1])
    TOT = NCH * 128

    idx16 = np.zeros((NCORES, TOT), np.int16)
    slotv = np.full((NCORES, TOT), -1.0, np.float32)
    sidx = idxv[order]
    sslot = slot[order]
    starts = np.concatenate([[0], np.cumsum(cnt.reshape(-1))]).astype(int)
    for c in range(NCORES):
        for g in range(GRP):
            base = choff[g] * 128
            for h, off in ((0, base), (1, base + CA[g] * 128)):
                k = (c * GRP + g) * 2 + h
                n = int(cnt[c, g, h])
                s0 = starts[k]
                idx16[c, off : off + n] = sidx[s0 : s0 + n]
                slotv[c, off : off + n] = sslot[s0 : s0 + n]

    idx_w = np.stack([np.tile(idx16[c].reshape(-1, 16).T, (8, 1)) for c in range(NCORES)])
    slot_cols = np.stack([slotv[c].reshape(NCH, 128).T for c in range(NCORES)])
    return idx_w, slot_cols, CA, CB, choff, NCH, TOT


def _build_program(CA, CB, choff, NCH, TOT):
    STAGE = os.environ.get("KSTAGE", "full")
    ORDER = ["deg", "t0", "ag0", "hop1", "aghop", "hop2", "layer0", "full"]
    LVL = ORDER.index(STAGE)
    nc = bacc.Bacc("TRN2", target_bir_lowering=False, debug=False, num_devices=NCORES)
    RG = [list(range(NCORES))]
    W16 = TOT // 16

    x_d = nc.dram_tensor("x_loc", [NPAD, F], FP, kind="ExternalInput")
    idx_d = nc.dram_tensor("idx_w", [128, W16], I16, kind="ExternalInput")
    slot_d = nc.dram_tensor("slot_cols", [128, NCH], FP, kind="ExternalInput")
    gslot_d = nc.dram_tensor("gslot", [128, GRP], FP, kind="ExternalInput")
    valid_d = nc.dram_tensor("valid", [128, GRP], FP, kind="ExternalInput")
    w_d = [nc.dram_tensor(f"w{l}", [(HOPS + 1) * F, F], FP, kind="ExternalInput")
           for l in range(HLAYERS + 1)]
    b_d = nc.dram_tensor("b_cols", [128, HLAYERS + 1], FP, kind="ExternalInput")
    wc_d = nc.dram_tensor("wc", [F, CLASSES], FP, kind="ExternalInput")
    bcr_d = nc.dram_tensor("bc_rep", [128, CLASSES], FP, kind="ExternalInput")
    out_d = nc.dram_tensor("out", [G, CLASSES], FP, kind="ExternalOutput")

    with tile.TileContext(nc) as tc:
        with (
            tc.tile_pool(name="const", bufs=1) as cp,
            tc.tile_pool(name="work", bufs=2) as wp,
            tc.tile_pool(name="psmm", bufs=3, space="PSUM") as pmm,
            tc.tile_pool(name="pstr", bufs=2, space="PSUM") as ptr,
            tc.tile_pool(name="psro", bufs=2, space="PSUM") as pro,
            tc.tile_pool(name="dram", bufs=1, space="DRAM") as dp,
        ):
            # ---- persistent tiles ----
            idx_t = cp.tile([128, W16], I16)
            slot_t = cp.tile([128, NCH], FP)
            gslot_t = cp.tile([128, GRP], FP)
            valid_t = cp.tile([128, GRP], FP)
            iota_t = cp.tile([128, 128], FP)
            ident_t = cp.tile([128, 128], FP)
            ones_t = cp.tile([128, 1], FP)
            normc_t = cp.tile([128, GRP], FP)
            w_t = [cp.tile([128, HOPS + 1, F], FP, name=f"w{l}_t", tag=f"w{l}")
                   for l in range(HLAYERS + 1)]
            b_t = cp.tile([128, HLAYERS + 1], FP)
            wc_t = cp.tile([F, CLASSES], FP)
            bcr_t = cp.tile([128, CLASSES], FP)
            f0T = cp.tile([128, GRP * 128], FP)   # feat-major [f, i] per group
            f1T = cp.tile([128, GRP * 128], FP)
            f2T = cp.tile([128, GRP * 128], FP)
            roacc_t = cp.tile([128, F + 1], FP)
            ro2_t = cp.tile([128, F + 1], FP)
            cnt_t = cp.tile([128, 1], FP)
            rcp_t = cp.tile([128, 1], FP)
            hg_t = cp.tile([128, F], FP)
            hgT_t = cp.tile([F, 128], FP)
            logit_t = cp.tile([128, CLASSES], FP)

            T_in = dp.tile([NT, F], FP)
            T_hop = dp.tile([NT, F], FP)
            ag_in = dp.tile([NPAD, F], FP)
            ar_in = dp.tile([128, F + 1], FP)
            ar_out = dp.tile([128, F + 1], FP)

            # ---- constants ----
            nc.sync.dma_start(idx_t[:], idx_d[:, :])
            nc.sync.dma_start(slot_t[:], slot_d[:, :])
            nc.sync.dma_start(gslot_t[:], gslot_d[:, :])
            nc.sync.dma_start(valid_t[:], valid_d[:, :])
            for l in range(HLAYERS + 1):
                for k in range(HOPS + 1):
                    nc.sync.dma_start(w_t[l][:, k, :], w_d[l][k * 128 : (k + 1) * 128, :])
            nc.sync.dma_start(b_t[:], b_d[:, :])
            nc.sync.dma_start(wc_t[:], wc_d[:, :])
            nc.sync.dma_start(bcr_t[:], bcr_d[:, :])

            nc.gpsimd.iota(iota_t[:], pattern=[[1, 128]], base=0, channel_multiplier=0,
                           allow_small_or_imprecise_dtypes=True)
            icol_t = cp.tile([128, 1], FP)
            nc.gpsimd.iota(icol_t[:], pattern=[[0, 1]], base=0, channel_multiplier=1,
                           allow_small_or_imprecise_dtypes=True)
            nc.vector.tensor_tensor(ident_t[:], icol_t[:].broadcast_to([128, 128]),
                                    iota_t[:], mybir.AluOpType.is_equal)
            nc.vector.memset(ones_t[:], 1.0)
            nc.vector.memset(roacc_t[:], 0.0)

            def bail():
                nc.vector.tensor_copy(logit_t[:], iota_t[:, :CLASSES])
                nc.sync.dma_start(out_d[:, :], logit_t[:])

            def onehot_all(g):
                """[128e, nch, 128j] one-hot tile for group g (one DVE op)."""
                nch = int(CA[g] + CB[g])
                c0 = int(choff[g])
                oh = wp.tile([128, int(max(CA + CB)), 128], FP, name="oh", tag="oh")
                nc.vector.tensor_tensor(
                    oh[:, :nch, :],
                    slot_t[:, c0 : c0 + nch].unsqueeze(2).broadcast_to([128, nch, 128]),
                    iota_t[:].unsqueeze(1).broadcast_to([128, nch, 128]),
                    mybir.AluOpType.is_equal,
                )
                return oh, nch

            # ---- degree / norm pass ----
            for g in range(GRP):
                oh, nch = onehot_all(g)
                dps = pmm.tile([128, 128], FP, name="dps", tag="mm")
                for c in range(nch):
                    nc.tensor.matmul(dps[:, 0:1], oh[:, c, :], ones_t[:],
                                     start=(c == 0), stop=(c == nch - 1))
                dmx = wp.tile([128, 1], FP, name="dmx", tag="dmx")
                nc.vector.tensor_scalar_max(dmx[:], dps[:, 0:1], 1.0)
                drc = wp.tile([128, 1], FP, name="drc", tag="drc")
                nc.vector.reciprocal(drc[:], dmx[:])
                nc.scalar.activation(normc_t[:, g : g + 1], drc[:],
                                     mybir.ActivationFunctionType.Sqrt)
            STOP = LVL <= ORDER.index("deg")
            if STOP:
                bail()

            # ---- T0 = x * norm ; f0T = x^T ----
            for g in range(GRP) if not STOP else []:
                gs = slice(g * 128, (g + 1) * 128)
                xt = wp.tile([128, F], FP, name="xt", tag="xt")
                nc.sync.dma_start(xt[:], x_d[gs, :])
                t0 = wp.tile([128, F], FP, name="t0", tag="tn")
                nc.vector.tensor_tensor(t0[:], xt[:],
                                        normc_t[:, g : g + 1].broadcast_to([128, F]),
                                        mybir.AluOpType.mult)
                nc.sync.dma_start(ag_in[gs, :], t0[:])
                pt = ptr.tile([128, 128], FP, name="pt", tag="tr")
                nc.tensor.transpose(pt[:], xt[:], ident_t[:])
                nc.vector.tensor_copy(f0T[:, gs], pt[:])
            if not STOP and LVL <= ORDER.index("t0"):
                bail()
                STOP = True
            if not STOP:
                nc.gpsimd.collective_compute(
                    "AllGather", mybir.AluOpType.bypass, replica_groups=RG,
                    ins=[ag_in.opt()], outs=[T_in.opt()])
            if not STOP and LVL <= ORDER.index("ag0"):
                bail()
                STOP = True

            def hop(src_tbl, fT, make_table):
                """One SpMM hop: gather -> one-hot segsum -> scale; optionally
                also emit next scaled table slice into ag_in."""
                KSUB = os.environ.get("KSUB", "full")
                for g in range(GRP):
                    gs = slice(g * 128, (g + 1) * 128)
                    ca, cb = int(CA[g]), int(CB[g])
                    nch = ca + cb
                    c0 = int(choff[g])
                    vb = wp.tile([128, int(max(CA + CB)), 128], FP, name="vb", tag="vb")
                    colA = c0 * 8
                    colB = colA + ca * 8
                    if KSUB in ("full", "gath"):
                        nc.gpsimd.dma_gather(
                            vb[:, 0:ca, :], src_tbl[:, :], idx_t[:, colA : colA + ca * 8],
                            ca * 128, ca * 128, F, single_packet=False)
                        nc.gpsimd.dma_gather(
                            vb[:, ca:nch, :], src_tbl[HALF:, :], idx_t[:, colB : colB + cb * 8],
                            cb * 128, cb * 128, F, single_packet=False)
                    else:
                        nc.vector.memset(vb[:], 0.0)
                    fn = wp.tile([128, F], FP, name="fn", tag="fn")
                    if KSUB == "gath":
                        nc.vector.tensor_copy(fn[:], vb[:, 0, :])
                    else:
                        oh, _ = onehot_all(g)
                        ps = pmm.tile([128, 128], FP, name="ps", tag="mm")
                        for c in range(nch):
                            nc.tensor.matmul(ps[:], oh[:, c, :], vb[:, c, :],
                                             start=(c == 0), stop=(c == nch - 1))
                        nc.vector.tensor_tensor(fn[:], ps[:],
                                                normc_t[:, g : g + 1].broadcast_to([128, F]),
                                                mybir.AluOpType.mult)
                    if make_table:
                        tn = wp.tile([128, F], FP, name="tn", tag="tn")
                        nc.vector.tensor_tensor(tn[:], fn[:],
                                                normc_t[:, g : g + 1].broadcast_to([128, F]),
                                                mybir.AluOpType.mult)
                        nc.sync.dma_start(ag_in[gs, :], tn[:])
                    pt = ptr.tile([128, 128], FP, name="pt2", tag="tr")
                    nc.tensor.transpose(pt[:], fn[:], ident_t[:])
                    nc.vector.tensor_copy(fT[:, gs], pt[:])

            for l in range(HLAYERS + 1) if not STOP else []:
                hop(T_in, f1T, make_table=True)
                if l == 0 and LVL <= ORDER.index("hop1"):
                    bail()
                    STOP = True
                    break
                nc.gpsimd.collective_compute(
                    "AllGather", mybir.AluOpType.bypass, replica_groups=RG,
                    ins=[ag_in.opt()], outs=[T_hop.opt()])
                if l == 0 and LVL <= ORDER.index("aghop"):
                    bail()
                    STOP = True
                    break
                hop(T_hop, f2T, make_table=False)
                if l == 0 and LVL <= ORDER.index("hop2"):
                    bail()
                    STOP = True
                    break
                fTs = [f0T, f1T, f2T]
                for g in range(GRP):
                    gs = slice(g * 128, (g + 1) * 128)
                    ph = pmm.tile([128, 128], FP, name="ph", tag="mm")
                    for k in range(HOPS + 1):
                        nc.tensor.matmul(ph[:], w_t[l][:, k, :], fTs[k][:, gs],
                                         start=(k == 0), stop=(k == HOPS))
                    nc.scalar.activation(f0T[:, gs], ph[:],
                                         mybir.ActivationFunctionType.Relu,
                                         bias=b_t[:, l : l + 1])
                    pt = ptr.tile([128, 128], FP, name="pt3", tag="tr")
                    nc.tensor.transpose(pt[:], f0T[:, gs], ident_t[:])
                    if l < HLAYERS:
                        tn = wp.tile([128, F], FP, name="tn2", tag="tn")
                        nc.vector.tensor_tensor(tn[:], pt[:],
                                                normc_t[:, g : g + 1].broadcast_to([128, F]),
                                                mybir.AluOpType.mult)
                        nc.sync.dma_start(ag_in[gs, :], tn[:])
                    else:
                        rr = wp.tile([128, F + 1], FP, name="rr", tag="rr")
                        nc.vector.tensor_copy(rr[:, 0:F], pt[:])
                        nc.vector.tensor_copy(rr[:, F : F + 1], valid_t[:, g : g + 1])
                        og = wp.tile([128, 128], FP, name="og", tag="og")
                        nc.vector.tensor_tensor(
                            og[:], gslot_t[:, g : g + 1].broadcast_to([128, 128]),
                            iota_t[:], mybir.AluOpType.is_equal)
                        pr = pro.tile([128, F + 1], FP, name="pr", tag="ro")
                        nc.tensor.matmul(pr[:], og[:], rr[:], start=True, stop=True)
                        nc.vector.tensor_tensor(roacc_t[:], roacc_t[:], pr[:],
                                                mybir.AluOpType.add)
                if l < HLAYERS:
                    nc.gpsimd.collective_compute(
                        "AllGather", mybir.AluOpType.bypass, replica_groups=RG,
                        ins=[ag_in.opt()], outs=[T_in.opt()])
                if l == 0 and LVL <= ORDER.index("layer0"):
                    bail()
                    STOP = True
                    break

            # ---- readout: all-reduce partial sums, mean, classify ----
            if not STOP:
                nc.sync.dma_start(ar_in[:, :], roacc_t[:])
                nc.gpsimd.collective_compute(
                    "AllReduce", mybir.AluOpType.add, replica_groups=RG,
                    ins=[ar_in.opt()], outs=[ar_out.opt()])
                nc.sync.dma_start(ro2_t[:], ar_out[:, :])
                nc.vector.tensor_scalar_max(cnt_t[:], ro2_t[:, F : F + 1], 1.0)
                nc.vector.reciprocal(rcp_t[:], cnt_t[:])
                nc.vector.tensor_tensor(hg_t[:], ro2_t[:, 0:F],
                                        rcp_t[:].broadcast_to([128, F]),
                                        mybir.AluOpType.mult)
                ptf = ptr.tile([128, 128], FP, name="ptf", tag="tr")
                nc.tensor.transpose(ptf[:], hg_t[:], ident_t[:])
                nc.vector.tensor_copy(hgT_t[:], ptf[:])
                plog = pro.tile([128, F + 1], FP, name="plog", tag="ro")
                nc.tensor.matmul(plog[:, 0:CLASSES], hgT_t[:], wc_t[:], start=True, stop=True)
                nc.vector.tensor_tensor(logit_t[:], plog[:, 0:CLASSES], bcr_t[:],
                                        mybir.AluOpType.add)
                nc.sync.dma_start(out_d[:, :], logit_t[:])

    nc.finalize()
    return nc


def kernel(x, src, dst, graph_ids, W0, b0, W1, b1, W2, b2, Wc, bc, **_):
    x = np.asarray(x, np.float32)
    graph_ids = np.asarray(graph_ids, np.int64)
    idx_w, slot_cols, CA, CB, choff, NCH, TOT = _prep_edges(np.asarray(src), np.asarray(dst))

    nc = _build_program(CA, CB, choff, NCH, TOT)

    in_maps = []
    Ws = [np.asarray(W0, np.float32), np.asarray(W1, np.float32), np.asarray(W2, np.float32)]
    bs = [np.asarray(b0, np.float32), np.asarray(b1, np.float32), np.asarray(b2, np.float32)]
    b_cols = np.stack(bs, axis=1).astype(np.float32)            # [128, 3]
    bc_rep = np.tile(np.asarray(bc, np.float32)[None, :], (128, 1))
    for c in range(NCORES):
        x_loc = np.zeros((NPAD, F), np.float32)
        x_loc[:PER] = x[c * PER : (c + 1) * PER]
        gsl = np.full(NPAD, -1.0, np.float32)
        gsl[:PER] = graph_ids[c * PER : (c + 1) * PER].astype(np.float32)
        val = np.zeros(NPAD, np.float32)
        val[:PER] = 1.0
        in_maps.append(dict(
            x_loc=x_loc,
            idx_w=idx_w[c],
            slot_cols=slot_cols[c],
            gslot=gsl.reshape(GRP, 128).T.copy(),
            valid=val.reshape(GRP, 128).T.copy(),
            w0=Ws[0], w1=Ws[1], w2=Ws[2],
            b_cols=b_cols, wc=np.asarray(Wc, np.float32),
            bc_rep=bc_rep,
        ))

    res = bass_utils.run_bass_kernel_spmd(nc, in_maps, core_ids=list(range(NCORES)))
    return np.asarray(res.results[0]["out"], np.float32)



# revision 6
# speedup vs baseline: 7.8531x; 7.8531x over previous
"""TAGConv GNN classifier on 8 Trainium2 NeuronCores.

Sharding: nodes split into 8 contiguous slices (6250/core, padded to 6272);
edges live on the core that owns their dst. Each hop: every core gathers
src rows from a replicated norm-prescaled bf16 node table in HBM
(dma_gather, int16 indices -> split-table trick), segment-sums them into
its dst slice with one-hot matmuls on TensorE (PSUM accumulation), rescales
by norm, and all-gathers its slice of the next table. Readout partial sums
per graph are all-reduced, then every core computes the (identical) logits.

Per-group work is uniform (chunk counts padded to the max) so each pass is
a single hardware For_i loop over the 49 dst groups -> ~40x fewer
instructions than full unrolling, which cuts per-call program
serialize/load overhead. Inputs are uploaded compactly (bf16 features and
weights, de-replicated int16 gather indices, uint8 slots/graph-ids) to cut
host->device transfer bytes.
"""
import os

import numpy as np
import ml_dtypes

import concourse.bass as bass
import concourse.bacc as bacc
import concourse.mybir as mybir
import concourse.tile as tile
from concourse import bass_utils
from concourse.bass import ds

N, E, G = 50000, 800000, 128
F = 128                      # IN_DIM == HID
CLASSES = 10
HOPS, HLAYERS = 2, 2         # 3 TAGConv layers total
NCORES = 8

PER = N // NCORES            # real nodes per core
GRP = (PER + 127) // 128     # dst groups of 128 per core
NPAD = GRP * 128             # padded nodes per core
NT = NCORES * NPAD           # padded total
HALF = NT // 2               # int16-safe split of the node table

FP = mybir.dt.float32
BF = mybir.dt.bfloat16
I16 = mybir.dt.int16
U8 = mybir.dt.uint8
NPBF = ml_dtypes.bfloat16


def _prep_edges(src, dst):
    """Per-core gather-index + slot tables with uniform chunks per group."""
    src = np.asarray(src).astype(np.int64)
    dst = np.asarray(dst).astype(np.int64)
    core = dst // PER
    local = dst - core * PER
    grp = local // 128
    slot = local % 128
    ps = (src // PER) * NPAD + (src % PER)          # padded global src id
    half = (ps >= HALF).astype(np.int64)
    idxv = ps - half * HALF                          # int16-safe index

    key = (core * GRP + grp) * 2 + half
    order = np.argsort(key, kind="stable")
    cnt = np.bincount(key, minlength=NCORES * GRP * 2).reshape(NCORES, GRP, 2)
    CAu = max(1, -(-int(cnt[:, :, 0].max()) // 128))
    CBu = max(1, -(-int(cnt[:, :, 1].max()) // 128))
    CH = CAu + CBu
    TOT = GRP * CH * 128

    idx16 = np.zeros((NCORES, TOT), np.int16)
    slotu = np.full((NCORES, TOT), 255, np.uint8)
    sidx = idxv[order]
    sslot = slot[order].astype(np.uint8)
    starts = np.concatenate([[0], np.cumsum(cnt.reshape(-1))]).astype(int)
    for c in range(NCORES):
        for g in range(GRP):
            base = g * CH * 128
            for h, off in ((0, base), (1, base + CAu * 128)):
                k = (c * GRP + g) * 2 + h
                n = int(cnt[c, g, h])
                s0 = starts[k]
                idx16[c, off : off + n] = sidx[s0 : s0 + n]
                slotu[c, off : off + n] = sslot[s0 : s0 + n]

    idx_c = np.ascontiguousarray(idx16.reshape(NCORES, -1, 16).transpose(0, 2, 1))
    slot_cols = np.ascontiguousarray(
        slotu.reshape(NCORES, GRP * CH, 128).transpose(0, 2, 1))
    return idx_c, slot_cols, CAu, CBu


def _build_program(CAu, CBu):
    STAGE = os.environ.get("KSTAGE", "full")
    ORDER = ["deg", "t0", "ag0", "hop1", "aghop", "hop2", "layer0", "full"]
    LVL = ORDER.index(STAGE)
    CH = CAu + CBu
    NCH = GRP * CH
    TOT = NCH * 128
    W16 = TOT // 16
    nc = bacc.Bacc("TRN2", target_bir_lowering=False, debug=False, num_devices=NCORES)
    RG = [list(range(NCORES))]

    x_d = nc.dram_tensor("x_bf", [NPAD, F], BF, kind="ExternalInput")
    idx_d = nc.dram_tensor("idx_c", [16, W16], I16, kind="ExternalInput")
    slot_d = nc.dram_tensor("slot_u8", [128, NCH], U8, kind="ExternalInput")
    gslot_d = nc.dram_tensor("gslot_u8", [128, GRP], U8, kind="ExternalInput")
    w_d = [nc.dram_tensor(f"w{l}", [(HOPS + 1) * F, F], BF, kind="ExternalInput")
           for l in range(HLAYERS + 1)]
    b_d = nc.dram_tensor("b_cols", [128, HLAYERS + 1], FP, kind="ExternalInput")
    wc_d = nc.dram_tensor("wc", [F, CLASSES], FP, kind="ExternalInput")
    bcr_d = nc.dram_tensor("bc_rep", [128, CLASSES], FP, kind="ExternalInput")
    out_d = nc.dram_tensor("out", [G, CLASSES], FP, kind="ExternalOutput")

    with tile.TileContext(nc) as tc:
        with (
            tc.tile_pool(name="const", bufs=1) as cp,
            tc.tile_pool(name="work", bufs=2) as wp,
            tc.tile_pool(name="psmm", bufs=3, space="PSUM") as pmm,
            tc.tile_pool(name="pstr", bufs=2, space="PSUM") as ptr,
            tc.tile_pool(name="psro", bufs=2, space="PSUM") as pro,
            tc.tile_pool(name="dram", bufs=1, space="DRAM") as dp,
        ):
            # ---- persistent tiles ----
            idx_t = cp.tile([128, W16], I16)
            slot8_t = cp.tile([128, NCH], U8)
            slot_t = cp.tile([128, NCH], BF)
            gslot8_t = cp.tile([128, GRP], U8)
            gslot_t = cp.tile([128, GRP], BF)
            iota_b = cp.tile([128, 128], BF)
            iota_f = cp.tile([128, 128], FP)
            ident_b = cp.tile([128, 128], BF)
            ident_f = cp.tile([128, 128], FP)
            ones_b = cp.tile([128, 1], BF)
            normc_t = cp.tile([128, GRP], FP)
            normb_t = cp.tile([128, GRP], BF)
            w_t = [cp.tile([128, HOPS + 1, F], BF, name=f"w{l}_t", tag=f"w{l}")
                   for l in range(HLAYERS + 1)]
            b_t = cp.tile([128, HLAYERS + 1], FP)
            wc_t = cp.tile([F, CLASSES], FP)
            bcr_t = cp.tile([128, CLASSES], FP)
            f0T = cp.tile([128, GRP * 128], BF)   # feat-major [f, i] per group
            f1T = cp.tile([128, GRP * 128], BF)
            f2T = cp.tile([128, GRP * 128], BF)
            roacc_t = cp.tile([128, F + 1], FP)
            ro2_t = cp.tile([128, F + 1], FP)
            cnt_t = cp.tile([128, 1], FP)
            rcp_t = cp.tile([128, 1], FP)
            hg_t = cp.tile([128, F], FP)
            hgT_t = cp.tile([F, 128], FP)
            logit_t = cp.tile([128, CLASSES], FP)

            T_in = dp.tile([NT, F], BF)
            T_hop = dp.tile([NT, F], BF)
            ag_in = dp.tile([NPAD, F], BF)
            ar_in = dp.tile([128, F + 1], FP)
            ar_out = dp.tile([128, F + 1], FP)

            # ---- constants ----
            for p in range(8):
                nc.sync.dma_start(idx_t[p * 16 : (p + 1) * 16, :], idx_d[:, :])
            nc.sync.dma_start(slot8_t[:], slot_d[:, :])
            nc.sync.dma_start(gslot8_t[:], gslot_d[:, :])
            nc.vector.tensor_copy(slot_t[:], slot8_t[:])
            nc.vector.tensor_copy(gslot_t[:], gslot8_t[:])
            for l in range(HLAYERS + 1):
                for k in range(HOPS + 1):
                    nc.sync.dma_start(w_t[l][:, k, :], w_d[l][k * 128 : (k + 1) * 128, :])
            nc.sync.dma_start(b_t[:], b_d[:, :])
            nc.sync.dma_start(wc_t[:], wc_d[:, :])
            nc.sync.dma_start(bcr_t[:], bcr_d[:, :])

            nc.gpsimd.iota(iota_f[:], pattern=[[1, 128]], base=0, channel_multiplier=0,
                           allow_small_or_imprecise_dtypes=True)
            nc.vector.tensor_copy(iota_b[:], iota_f[:])
            icol_t = cp.tile([128, 1], FP)
            nc.gpsimd.iota(icol_t[:], pattern=[[0, 1]], base=0, channel_multiplier=1,
                           allow_small_or_imprecise_dtypes=True)
            nc.vector.tensor_tensor(ident_f[:], icol_t[:].broadcast_to([128, 128]),
                                    iota_f[:], mybir.AluOpType.is_equal)
            nc.vector.tensor_copy(ident_b[:], ident_f[:])
            nc.vector.memset(ones_b[:], 1.0)
            nc.vector.memset(roacc_t[:], 0.0)

            def bail():
                nc.vector.tensor_copy(logit_t[:], iota_f[:, :CLASSES])
                nc.sync.dma_start(out_d[:, :], logit_t[:])

            def onehot(g):
                """[128e, CH, 128j] one-hot tile for group g (one DVE op)."""
                oh = wp.tile([128, CH, 128], BF, name="oh", tag="oh")
                nc.vector.tensor_tensor(
                    oh[:, :, :],
                    slot_t[:, ds(g * CH, CH)].unsqueeze(2).broadcast_to([128, CH, 128]),
                    iota_b[:].unsqueeze(1).broadcast_to([128, CH, 128]),
                    mybir.AluOpType.is_equal,
                )
                return oh

            # ---- degree / norm pass ----
            with tc.For_i(0, GRP, 1) as g:
                oh = onehot(g)
                dps = pmm.tile([128, 128], FP, name="dps", tag="mm")
                for c in range(CH):
                    nc.tensor.matmul(dps[:, 0:1], oh[:, c, :], ones_b[:],
                                     start=(c == 0), stop=(c == CH - 1))
                dmx = wp.tile([128, 1], FP, name="dmx", tag="dmx")
                nc.vector.tensor_scalar_max(dmx[:], dps[:, 0:1], 1.0)
                drc = wp.tile([128, 1], FP, name="drc", tag="drc")
                nc.vector.reciprocal(drc[:], dmx[:])
                nc.scalar.activation(normc_t[:, ds(g, 1)], drc[:],
                                     mybir.ActivationFunctionType.Sqrt)
            nc.vector.tensor_copy(normb_t[:], normc_t[:])
            STOP = LVL <= ORDER.index("deg")
            if STOP:
                bail()

            # ---- T0 = x * norm ; f0T = x^T ----
            if not STOP:
                with tc.For_i(0, GRP, 1) as g:
                    xt = wp.tile([128, F], BF, name="xt", tag="xt")
                    nc.sync.dma_start(xt[:], x_d[ds(g * 128, 128), :])
                    t0 = wp.tile([128, F], BF, name="t0", tag="tn")
                    nc.vector.tensor_tensor(
                        t0[:], xt[:], normb_t[:, ds(g, 1)].broadcast_to([128, F]),
                        mybir.AluOpType.mult)
                    nc.sync.dma_start(ag_in[ds(g * 128, 128), :], t0[:])
                    pt = ptr.tile([128, 128], BF, name="pt", tag="tr")
                    nc.tensor.transpose(pt[:], xt[:], ident_b[:])
                    nc.vector.tensor_copy(f0T[:, ds(g * 128, 128)], pt[:])
            if not STOP and LVL <= ORDER.index("t0"):
                bail()
                STOP = True
            if not STOP:
                nc.gpsimd.collective_compute(
                    "AllGather", mybir.AluOpType.bypass, replica_groups=RG,
                    ins=[ag_in.opt()], outs=[T_in.opt()])
            if not STOP and LVL <= ORDER.index("ag0"):
                bail()
                STOP = True

            def hop(src_tbl, fT, make_table):
                """One SpMM hop: gather -> one-hot segsum -> scale; optionally
                also emit next scaled table slice into ag_in."""
                with tc.For_i(0, GRP, 1) as g:
                    vb = wp.tile([128, CH, 128], BF, name="vb", tag="vb")
                    nc.gpsimd.dma_gather(
                        vb[:, 0:CAu, :], src_tbl[:, :],
                        idx_t[:, ds(g * CH * 8, CAu * 8)],
                        CAu * 128, CAu * 128, F, single_packet=False)
                    nc.gpsimd.dma_gather(
                        vb[:, CAu:CH, :], src_tbl[HALF:, :],
                        idx_t[:, ds(g * CH * 8 + CAu * 8, CBu * 8)],
                        CBu * 128, CBu * 128, F, single_packet=False)
                    oh = onehot(g)
                    ps = pmm.tile([128, 128], FP, name="ps", tag="mm")
                    for c in range(CH):
                        nc.tensor.matmul(ps[:], oh[:, c, :], vb[:, c, :],
                                         start=(c == 0), stop=(c == CH - 1))
                    fn = wp.tile([128, F], BF, name="fn", tag="fn")
                    nc.vector.tensor_tensor(
                        fn[:], ps[:], normc_t[:, ds(g, 1)].broadcast_to([128, F]),
                        mybir.AluOpType.mult)
                    if make_table:
                        tn = wp.tile([128, F], BF, name="tn", tag="tn")
                        nc.vector.tensor_tensor(
                            tn[:], fn[:], normb_t[:, ds(g, 1)].broadcast_to([128, F]),
                            mybir.AluOpType.mult)
                        nc.sync.dma_start(ag_in[ds(g * 128, 128), :], tn[:])
                    pt = ptr.tile([128, 128], BF, name="pt2", tag="tr")
                    nc.tensor.transpose(pt[:], fn[:], ident_b[:])
                    nc.vector.tensor_copy(fT[:, ds(g * 128, 128)], pt[:])

            for l in range(HLAYERS + 1) if not STOP else []:
                hop(T_in, f1T, make_table=True)
                if l == 0 and LVL <= ORDER.index("hop1"):
                    bail()
                    STOP = True
                    break
                nc.gpsimd.collective_compute(
                    "AllGather", mybir.AluOpType.bypass, replica_groups=RG,
                    ins=[ag_in.opt()], outs=[T_hop.opt()])
                if l == 0 and LVL <= ORDER.index("aghop"):
                    bail()
                    STOP = True
                    break
                hop(T_hop, f2T, make_table=False)
                if l == 0 and LVL <= ORDER.index("hop2"):
                    bail()
                    STOP = True
                    break
                fTs = [f0T, f1T, f2T]
                with tc.For_i(0, GRP, 1) as g:
                    ph = pmm.tile([128, 128], FP, name="ph", tag="mm")
                    for k in range(HOPS + 1):
                        nc.tensor.matmul(ph[:], w_t[l][:, k, :],
                                         fTs[k][:, ds(g * 128, 128)],
                                         start=(k == 0), stop=(k == HOPS))
                    act = wp.tile([128, 128], BF, name="act", tag="act")
                    nc.scalar.activation(act[:], ph[:],
                                         mybir.ActivationFunctionType.Relu,
                                         bias=b_t[:, l : l + 1])
                    nc.vector.tensor_copy(f0T[:, ds(g * 128, 128)], act[:])
                    pt = ptr.tile([128, 128], BF, name="pt3", tag="tr")
                    nc.tensor.transpose(pt[:], act[:], ident_b[:])
                    if l < HLAYERS:
                        tn = wp.tile([128, F], BF, name="tn2", tag="tn")
                        nc.vector.tensor_tensor(
                            tn[:], pt[:], normb_t[:, ds(g, 1)].broadcast_to([128, F]),
                            mybir.AluOpType.mult)
                        nc.sync.dma_start(ag_in[ds(g * 128, 128), :], tn[:])
                    else:
                        rr = wp.tile([128, F + 1], BF, name="rr", tag="rr")
                        nc.vector.tensor_copy(rr[:, 0:F], pt[:])
                        nc.vector.tensor_copy(rr[:, F : F + 1], ones_b[:])
                        og = wp.tile([128, 128], BF, name="og", tag="og")
                        nc.vector.tensor_tensor(
                            og[:], gslot_t[:, ds(g, 1)].broadcast_to([128, 128]),
                            iota_b[:], mybir.AluOpType.is_equal)
                        pr = pro.tile([128, F + 1], FP, name="pr", tag="ro")
                        nc.tensor.matmul(pr[:], og[:], rr[:], start=True, stop=True)
                        nc.vector.tensor_tensor(roacc_t[:], roacc_t[:], pr[:],
                                                mybir.AluOpType.add)
                if l < HLAYERS:
                    nc.gpsimd.collective_compute(
                        "AllGather", mybir.AluOpType.bypass, replica_groups=RG,
                        ins=[ag_in.opt()], outs=[T_in.opt()])
                if l == 0 and LVL <= ORDER.index("layer0"):
                    bail()
                    STOP = True
                    break

            # ---- readout: all-reduce partial sums, mean, classify ----
            if not STOP:
                nc.sync.dma_start(ar_in[:, :], roacc_t[:])
                nc.gpsimd.collective_compute(
                    "AllReduce", mybir.AluOpType.add, replica_groups=RG,
                    ins=[ar_in.opt()], outs=[ar_out.opt()])
                nc.sync.dma_start(ro2_t[:], ar_out[:, :])
                nc.vector.tensor_scalar_max(cnt_t[:], ro2_t[:, F : F + 1], 1.0)
                nc.vector.reciprocal(rcp_t[:], cnt_t[:])
                nc.vector.tensor_tensor(hg_t[:], ro2_t[:, 0:F],
                                        rcp_t[:].broadcast_to([128, F]),
                                        mybir.AluOpType.mult)
                ptf = ptr.tile([128, 128], FP, name="ptf", tag="tr")
                nc.tensor.transpose(ptf[:], hg_t[:], ident_f[:])
                nc.vector.tensor_copy(hgT_t[:], ptf[:])
                plog = pro.tile([128, F + 1], FP, name="plog", tag="ro")
                nc.tensor.matmul(plog[:, 0:CLASSES], hgT_t[:], wc_t[:], start=True, stop=True)
                nc.vector.tensor_tensor(logit_t[:], plog[:, 0:CLASSES], bcr_t[:],
                                        mybir.AluOpType.add)
                nc.sync.dma_start(out_d[:, :], logit_t[:])

    nc.finalize()
    return nc


def _make_in_maps(x, graph_ids, Ws, bs, Wc, bc, idx_c, slot_cols):
    b_cols = np.stack(bs, axis=1).astype(np.float32)            # [128, 3]
    bc_rep = np.tile(np.asarray(bc, np.float32)[None, :], (128, 1))
    Wbf = [np.asarray(w, np.float32).astype(NPBF) for w in Ws]
    in_maps = []
    for c in range(NCORES):
        x_loc = np.zeros((NPAD, F), NPBF)
        x_loc[:PER] = x[c * PER : (c + 1) * PER].astype(NPBF)
        gsl = np.full(NPAD, 255, np.uint8)
        gsl[:PER] = graph_ids[c * PER : (c + 1) * PER].astype(np.uint8)
        in_maps.append(dict(
            x_bf=x_loc,
            idx_c=idx_c[c],
            slot_u8=slot_cols[c],
            gslot_u8=np.ascontiguousarray(gsl.reshape(GRP, 128).T),
            w0=Wbf[0], w1=Wbf[1], w2=Wbf[2],
            b_cols=b_cols, wc=np.asarray(Wc, np.float32),
            bc_rep=bc_rep,
        ))
    return in_maps


def kernel(x, src, dst, graph_ids, W0, b0, W1, b1, W2, b2, Wc, bc, **_):
    x = np.asarray(x, np.float32)
    graph_ids = np.asarray(graph_ids, np.int64)
    idx_c, slot_cols, CAu, CBu = _prep_edges(src, dst)
    nc = _build_program(CAu, CBu)
    in_maps = _make_in_maps(
        x, graph_ids,
        [np.asarray(W0), np.asarray(W1), np.asarray(W2)],
        [np.asarray(b0, np.float32), np.asarray(b1, np.float32),
         np.asarray(b2, np.float32)],
        Wc, bc, idx_c, slot_cols)
    res = bass_utils.run_bass_kernel_spmd(nc, in_maps, core_ids=list(range(NCORES)))
    return np.asarray(res.results[0]["out"], np.float32)


# revision 15
# speedup vs baseline: 8.4688x; 1.0784x over previous
"""TAGConv GNN classifier on 8 Trainium2 NeuronCores.

Sharding: nodes split into 8 contiguous slices (6250/core, padded to 6272);
edges live on the core that owns their dst. Each hop: every core gathers
src rows from a replicated norm-prescaled bf16 node table in HBM
(dma_gather, int16 indices -> split-table trick), segment-sums them into
its dst slice with one-hot matmuls on TensorE (PSUM accumulation), rescales
by norm, and all-gathers its slice of the next table. Readout partial sums
per graph are all-reduced, then every core computes the (identical) logits.

Per-group work is uniform (chunk counts padded to the max) so each pass is
a single hardware For_i loop over the 49 dst groups -> ~40x fewer
instructions than full unrolling, which cuts per-call program
serialize/load overhead. Inputs are uploaded compactly (bf16 features and
weights, de-replicated int16 gather indices, uint8 slots/graph-ids) to cut
host->device transfer bytes.
"""
import os

import numpy as np
import ml_dtypes

import concourse.bass as bass
import concourse.bacc as bacc
import concourse.mybir as mybir
import concourse.tile as tile
from concourse import bass_utils
from concourse.bass import ds

N, E, G = 50000, 800000, 128
F = 128                      # IN_DIM == HID
CLASSES = 10
HOPS, HLAYERS = 2, 2         # 3 TAGConv layers total
NCORES = 8

PER = N // NCORES            # real nodes per core
GRP = (PER + 127) // 128     # dst groups of 128 per core
NPAD = GRP * 128             # padded nodes per core
NT = NCORES * NPAD           # padded total
HALF = NT // 2               # int16-safe split of the node table

FP = mybir.dt.float32
BF = mybir.dt.bfloat16
I16 = mybir.dt.int16
U8 = mybir.dt.uint8
NPBF = ml_dtypes.bfloat16


def _prep_edges(src, dst):
    """Per-core gather-index + slot tables with uniform chunks per group."""
    src = np.asarray(src).astype(np.int64)
    dst = np.asarray(dst).astype(np.int64)
    core = dst // PER
    local = dst - core * PER
    grp = local // 128
    slot = local % 128
    ps = (src // PER) * NPAD + (src % PER)          # padded global src id
    half = (ps >= HALF).astype(np.int64)
    idxv = ps - half * HALF                          # int16-safe index

    key = (core * GRP + grp) * 2 + half
    order = np.argsort(key, kind="stable")
    cnt = np.bincount(key, minlength=NCORES * GRP * 2).reshape(NCORES, GRP, 2)
    CAu = max(1, -(-int(cnt[:, :, 0].max()) // 128))
    CBu = max(1, -(-int(cnt[:, :, 1].max()) // 128))
    CH = CAu + CBu
    TOT = GRP * CH * 128

    idx16 = np.zeros((NCORES, TOT), np.int16)
    slotu = np.full((NCORES, TOT), 255, np.uint8)
    sidx = idxv[order]
    sslot = slot[order].astype(np.uint8)
    starts = np.concatenate([[0], np.cumsum(cnt.reshape(-1))]).astype(int)
    for c in range(NCORES):
        for g in range(GRP):
            base = g * CH * 128
            for h, off in ((0, base), (1, base + CAu * 128)):
                k = (c * GRP + g) * 2 + h
                n = int(cnt[c, g, h])
                s0 = starts[k]
                idx16[c, off : off + n] = sidx[s0 : s0 + n]
                slotu[c, off : off + n] = sslot[s0 : s0 + n]

    idx_c = np.ascontiguousarray(idx16.reshape(NCORES, -1, 16).transpose(0, 2, 1))
    slot_cols = np.ascontiguousarray(
        slotu.reshape(NCORES, GRP * CH, 128).transpose(0, 2, 1))
    return idx_c, slot_cols, CAu, CBu


def _build_program(CAu, CBu):
    STAGE = os.environ.get("KSTAGE", "full")
    ORDER = ["deg", "t0", "ag0", "hop1", "aghop", "hop2", "layer0", "full"]
    LVL = ORDER.index(STAGE)
    CH = CAu + CBu
    NCH = GRP * CH
    TOT = NCH * 128
    W16 = TOT // 16
    nc = bacc.Bacc("TRN2", target_bir_lowering=False, debug=False, num_devices=NCORES)
    RG = [list(range(NCORES))]

    I8 = mybir.dt.int8
    # misc layout (fp32 columns): [0:3]=b_cols, [3:13]=wc rows 0..127? no ->
    # wc is [F, CLASSES] so it packs as 10 columns; [13:23]=bc_rep,
    # [23:23+GRP]=x row scales (slot-major), [23+GRP:23+2*GRP]=gslot as fp32.
    MC_B, MC_WC, MC_BC = 0, 3, 3 + CLASSES
    MC_XS = MC_BC + CLASSES
    MC_GS = MC_XS + GRP
    MCOLS = MC_GS + GRP
    x_d = nc.dram_tensor("x_i8", [NPAD, F], I8, kind="ExternalInput")
    idx_d = nc.dram_tensor("idx_c", [16, W16], I16, kind="ExternalInput")
    slot_d = nc.dram_tensor("slot_u8", [128, NCH], U8, kind="ExternalInput")
    w_d = nc.dram_tensor("w_bf", [(HLAYERS + 1) * (HOPS + 1) * F, F], BF,
                         kind="ExternalInput")
    misc_d = nc.dram_tensor("misc", [128, MCOLS], FP, kind="ExternalInput")
    out_d = nc.dram_tensor("out", [G, CLASSES], FP, kind="ExternalOutput")

    with tile.TileContext(nc) as tc:
        with (
            tc.tile_pool(name="const", bufs=1) as cp,
            tc.tile_pool(name="work", bufs=2) as wp,
            tc.tile_pool(name="psmm", bufs=3, space="PSUM") as pmm,
            tc.tile_pool(name="pstr", bufs=2, space="PSUM") as ptr,
            tc.tile_pool(name="psro", bufs=2, space="PSUM") as pro,
            tc.tile_pool(name="dram", bufs=1, space="DRAM") as dp,
        ):
            # ---- persistent tiles ----
            idx_t = cp.tile([128, W16], I16)
            slot8_t = cp.tile([128, NCH], U8)
            slot_t = cp.tile([128, NCH], BF)
            misc_t = cp.tile([128, MCOLS], FP)
            gslot_t = cp.tile([128, GRP], BF)
            xsb_t = cp.tile([128, GRP], BF)
            iota_b = cp.tile([128, 128], BF)
            iota_f = cp.tile([128, 128], FP)
            ident_b = cp.tile([128, 128], BF)
            ident_f = cp.tile([128, 128], FP)
            ones_b = cp.tile([128, 1], BF)
            normc_t = cp.tile([128, GRP], FP)
            normb_t = cp.tile([128, GRP], BF)
            w_t = [cp.tile([128, HOPS + 1, F], BF, name=f"w{l}_t", tag=f"w{l}")
                   for l in range(HLAYERS + 1)]
            f0T = cp.tile([128, GRP * 128], BF)   # feat-major [f, i] per group
            f1T = cp.tile([128, GRP * 128], BF)
            f2T = cp.tile([128, GRP * 128], BF)
            roacc_t = cp.tile([128, F + 1], FP)
            ro2_t = cp.tile([128, F + 1], FP)
            cnt_t = cp.tile([128, 1], FP)
            rcp_t = cp.tile([128, 1], FP)
            hg_t = cp.tile([128, F], FP)
            hgT_t = cp.tile([F, 128], FP)
            logit_t = cp.tile([128, CLASSES], FP)

            T_in = dp.tile([NT, F], BF)
            T_hop = dp.tile([NT, F], BF)
            ag_in = dp.tile([NPAD, F], BF)
            ar_in = dp.tile([128, F + 1], FP)
            ar_out = dp.tile([128, F + 1], FP)

            # ---- constants ----
            for p in range(8):
                nc.sync.dma_start(idx_t[p * 16 : (p + 1) * 16, :], idx_d[:, :])
            nc.sync.dma_start(slot8_t[:], slot_d[:, :])
            nc.sync.dma_start(misc_t[:], misc_d[:, :])
            nc.vector.tensor_copy(slot_t[:], slot8_t[:])
            nc.vector.tensor_copy(gslot_t[:], misc_t[:, MC_GS : MC_GS + GRP])
            nc.vector.tensor_copy(xsb_t[:], misc_t[:, MC_XS : MC_XS + GRP])
            for l in range(HLAYERS + 1):
                for k in range(HOPS + 1):
                    r0 = (l * (HOPS + 1) + k) * 128
                    nc.sync.dma_start(w_t[l][:, k, :], w_d[r0 : r0 + 128, :])

            nc.gpsimd.iota(iota_f[:], pattern=[[1, 128]], base=0, channel_multiplier=0,
                           allow_small_or_imprecise_dtypes=True)
            nc.vector.tensor_copy(iota_b[:], iota_f[:])
            icol_t = cp.tile([128, 1], FP)
            nc.gpsimd.iota(icol_t[:], pattern=[[0, 1]], base=0, channel_multiplier=1,
                           allow_small_or_imprecise_dtypes=True)
            nc.vector.tensor_tensor(ident_f[:], icol_t[:].broadcast_to([128, 128]),
                                    iota_f[:], mybir.AluOpType.is_equal)
            nc.vector.tensor_copy(ident_b[:], ident_f[:])
            nc.vector.memset(ones_b[:], 1.0)
            nc.vector.memset(roacc_t[:], 0.0)

            def bail():
                nc.vector.tensor_copy(logit_t[:], iota_f[:, :CLASSES])
                nc.sync.dma_start(out_d[:, :], logit_t[:])

            def onehot(g):
                """[128e, CH, 128j] one-hot tile for group g (one DVE op)."""
                oh = wp.tile([128, CH, 128], BF, name="oh", tag="oh")
                nc.vector.tensor_tensor(
                    oh[:, :, :],
                    slot_t[:, ds(g * CH, CH)].unsqueeze(2).broadcast_to([128, CH, 128]),
                    iota_b[:].unsqueeze(1).broadcast_to([128, CH, 128]),
                    mybir.AluOpType.is_equal,
                )
                return oh

            # ---- degree / norm pass ----
            with tc.For_i(0, GRP, 1) as g:
                oh = onehot(g)
                dps = pmm.tile([128, 128], FP, name="dps", tag="mm")
                for c in range(CH):
                    nc.tensor.matmul(dps[:, 0:1], oh[:, c, :], ones_b[:],
                                     start=(c == 0), stop=(c == CH - 1))
                dmx = wp.tile([128, 1], FP, name="dmx", tag="dmx")
                nc.vector.tensor_scalar_max(dmx[:], dps[:, 0:1], 1.0)
                drc = wp.tile([128, 1], FP, name="drc", tag="drc")
                nc.vector.reciprocal(drc[:], dmx[:])
                nc.scalar.activation(normc_t[:, ds(g, 1)], drc[:],
                                     mybir.ActivationFunctionType.Sqrt)
            nc.vector.tensor_copy(normb_t[:], normc_t[:])
            STOP = LVL <= ORDER.index("deg")
            if STOP:
                bail()

            # ---- T0 = x * norm ; f0T = x^T ----
            if not STOP:
                with tc.For_i(0, GRP, 1) as g:
                    x8 = wp.tile([128, F], I8, name="x8", tag="x8")
                    nc.sync.dma_start(x8[:], x_d[ds(g * 128, 128), :])
                    xb = wp.tile([128, F], BF, name="xb", tag="xb")
                    nc.vector.tensor_copy(xb[:], x8[:])
                    xt = wp.tile([128, F], BF, name="xt", tag="xt")
                    nc.vector.tensor_tensor(
                        xt[:], xb[:], xsb_t[:, ds(g, 1)].broadcast_to([128, F]),
                        mybir.AluOpType.mult)
                    t0 = wp.tile([128, F], BF, name="t0", tag="tn")
                    nc.vector.tensor_tensor(
                        t0[:], xt[:], normb_t[:, ds(g, 1)].broadcast_to([128, F]),
                        mybir.AluOpType.mult)
                    nc.sync.dma_start(ag_in[ds(g * 128, 128), :], t0[:])
                    pt = ptr.tile([128, 128], BF, name="pt", tag="tr")
                    nc.tensor.transpose(pt[:], xt[:], ident_b[:])
                    nc.vector.tensor_copy(f0T[:, ds(g * 128, 128)], pt[:])
            if not STOP and LVL <= ORDER.index("t0"):
                bail()
                STOP = True
            if not STOP:
                nc.gpsimd.collective_compute(
                    "AllGather", mybir.AluOpType.bypass, replica_groups=RG,
                    ins=[ag_in.opt()], outs=[T_in.opt()])
            if not STOP and LVL <= ORDER.index("ag0"):
                bail()
                STOP = True

            def hop(src_tbl, fT, make_table):
                """One SpMM hop: gather -> one-hot segsum -> scale; optionally
                also emit next scaled table slice into ag_in."""
                with tc.For_i(0, GRP, 1) as g:
                    vb = wp.tile([128, CH, 128], BF, name="vb", tag="vb")
                    nc.gpsimd.dma_gather(
                        vb[:, 0:CAu, :], src_tbl[:, :],
                        idx_t[:, ds(g * CH * 8, CAu * 8)],
                        CAu * 128, CAu * 128, F, single_packet=False)
                    nc.gpsimd.dma_gather(
                        vb[:, CAu:CH, :], src_tbl[HALF:, :],
                        idx_t[:, ds(g * CH * 8 + CAu * 8, CBu * 8)],
                        CBu * 128, CBu * 128, F, single_packet=False)
                    oh = onehot(g)
                    ps = pmm.tile([128, 128], FP, name="ps", tag="mm")
                    for c in range(CH):
                        nc.tensor.matmul(ps[:], oh[:, c, :], vb[:, c, :],
                                         start=(c == 0), stop=(c == CH - 1))
                    fn = wp.tile([128, F], BF, name="fn", tag="fn")
                    nc.vector.tensor_tensor(
                        fn[:], ps[:], normc_t[:, ds(g, 1)].broadcast_to([128, F]),
                        mybir.AluOpType.mult)
                    if make_table:
                        tn = wp.tile([128, F], BF, name="tn", tag="tn")
                        nc.vector.tensor_tensor(
                            tn[:], fn[:], normb_t[:, ds(g, 1)].broadcast_to([128, F]),
                            mybir.AluOpType.mult)
                        nc.sync.dma_start(ag_in[ds(g * 128, 128), :], tn[:])
                    pt = ptr.tile([128, 128], BF, name="pt2", tag="tr")
                    nc.tensor.transpose(pt[:], fn[:], ident_b[:])
                    nc.vector.tensor_copy(fT[:, ds(g * 128, 128)], pt[:])

            for l in range(HLAYERS + 1) if not STOP else []:
                hop(T_in, f1T, make_table=True)
                if l == 0 and LVL <= ORDER.index("hop1"):
                    bail()
                    STOP = True
                    break
                nc.gpsimd.collective_compute(
                    "AllGather", mybir.AluOpType.bypass, replica_groups=RG,
                    ins=[ag_in.opt()], outs=[T_hop.opt()])
                if l == 0 and LVL <= ORDER.index("aghop"):
                    bail()
                    STOP = True
                    break
                hop(T_hop, f2T, make_table=False)
                if l == 0 and LVL <= ORDER.index("hop2"):
                    bail()
                    STOP = True
                    break
                fTs = [f0T, f1T, f2T]
                with tc.For_i(0, GRP, 1) as g:
                    ph = pmm.tile([128, 128], FP, name="ph", tag="mm")
                    for k in range(HOPS + 1):
                        nc.tensor.matmul(ph[:], w_t[l][:, k, :],
                                         fTs[k][:, ds(g * 128, 128)],
                                         start=(k == 0), stop=(k == HOPS))
                    act = wp.tile([128, 128], BF, name="act", tag="act")
                    nc.scalar.activation(act[:], ph[:],
                                         mybir.ActivationFunctionType.Relu,
                                         bias=misc_t[:, MC_B + l : MC_B + l + 1])
                    nc.vector.tensor_copy(f0T[:, ds(g * 128, 128)], act[:])
                    pt = ptr.tile([128, 128], BF, name="pt3", tag="tr")
                    nc.tensor.transpose(pt[:], act[:], ident_b[:])
                    if l < HLAYERS:
                        tn = wp.tile([128, F], BF, name="tn2", tag="tn")
                        nc.vector.tensor_tensor(
                            tn[:], pt[:], normb_t[:, ds(g, 1)].broadcast_to([128, F]),
                            mybir.AluOpType.mult)
                        nc.sync.dma_start(ag_in[ds(g * 128, 128), :], tn[:])
                    else:
                        rr = wp.tile([128, F + 1], BF, name="rr", tag="rr")
                        nc.vector.tensor_copy(rr[:, 0:F], pt[:])
                        nc.vector.tensor_copy(rr[:, F : F + 1], ones_b[:])
                        og = wp.tile([128, 128], BF, name="og", tag="og")
                        nc.vector.tensor_tensor(
                            og[:], gslot_t[:, ds(g, 1)].broadcast_to([128, 128]),
                            iota_b[:], mybir.AluOpType.is_equal)
                        pr = pro.tile([128, F + 1], FP, name="pr", tag="ro")
                        nc.tensor.matmul(pr[:], og[:], rr[:], start=True, stop=True)
                        nc.vector.tensor_tensor(roacc_t[:], roacc_t[:], pr[:],
                                                mybir.AluOpType.add)
                if l < HLAYERS:
                    nc.gpsimd.collective_compute(
                        "AllGather", mybir.AluOpType.bypass, replica_groups=RG,
                        ins=[ag_in.opt()], outs=[T_in.opt()])
                if l == 0 and LVL <= ORDER.index("layer0"):
                    bail()
                    STOP = True
                    break

            # ---- readout: all-reduce partial sums, mean, classify ----
            if not STOP:
                nc.sync.dma_start(ar_in[:, :], roacc_t[:])
                nc.gpsimd.collective_compute(
                    "AllReduce", mybir.AluOpType.add, replica_groups=RG,
                    ins=[ar_in.opt()], outs=[ar_out.opt()])
                nc.sync.dma_start(ro2_t[:], ar_out[:, :])
                nc.vector.tensor_scalar_max(cnt_t[:], ro2_t[:, F : F + 1], 1.0)
                nc.vector.reciprocal(rcp_t[:], cnt_t[:])
                nc.vector.tensor_tensor(hg_t[:], ro2_t[:, 0:F],
                                        rcp_t[:].broadcast_to([128, F]),
                                        mybir.AluOpType.mult)
                ptf = ptr.tile([128, 128], FP, name="ptf", tag="tr")
                nc.tensor.transpose(ptf[:], hg_t[:], ident_f[:])
                nc.vector.tensor_copy(hgT_t[:], ptf[:])
                plog = pro.tile([128, F + 1], FP, name="plog", tag="ro")
                nc.tensor.matmul(plog[:, 0:CLASSES], hgT_t[:],
                                 misc_t[:, MC_WC : MC_WC + CLASSES],
                                 start=True, stop=True)
                nc.vector.tensor_tensor(logit_t[:], plog[:, 0:CLASSES],
                                        misc_t[:, MC_BC : MC_BC + CLASSES],
                                        mybir.AluOpType.add)
                nc.sync.dma_start(out_d[:, :], logit_t[:])

    nc.finalize()
    return nc


def _make_in_maps(x, graph_ids, Ws, bs, Wc, bc, idx_c, slot_cols):
    b_cols = np.stack(bs, axis=1).astype(np.float32)            # [128, 3]
    bc_rep = np.tile(np.asarray(bc, np.float32)[None, :], (128, 1))
    w_bf = np.concatenate([np.asarray(w, np.float32) for w in Ws], axis=0).astype(NPBF)
    wc_f = np.asarray(Wc, np.float32)
    # per-node int8 quantization of x
    x_full8 = np.zeros((N, F), np.int8)
    xs_full = np.ones(N, np.float32)
    amax = np.abs(x).max(axis=1)
    nz = amax > 0
    xs_full[nz] = amax[nz] / 127.0
    x_full8 = np.clip(np.round(x / xs_full[:, None]), -127, 127).astype(np.int8)
    in_maps = []
    for c in range(NCORES):
        x_loc = np.zeros((NPAD, F), np.int8)
        x_loc[:PER] = x_full8[c * PER : (c + 1) * PER]
        xs = np.ones(NPAD, np.float32)
        xs[:PER] = xs_full[c * PER : (c + 1) * PER]
        gsl = np.full(NPAD, 255.0, np.float32)
        gsl[:PER] = graph_ids[c * PER : (c + 1) * PER].astype(np.float32)
        misc = np.concatenate([
            b_cols, wc_f, bc_rep,
            np.ascontiguousarray(xs.reshape(GRP, 128).T),
            np.ascontiguousarray(gsl.reshape(GRP, 128).T),
        ], axis=1).astype(np.float32)
        in_maps.append(dict(
            x_i8=x_loc,
            idx_c=idx_c[c],
            slot_u8=slot_cols[c],
            w_bf=w_bf,
            misc=misc,
        ))
    return in_maps


def kernel(x, src, dst, graph_ids, W0, b0, W1, b1, W2, b2, Wc, bc, **_):
    x = np.asarray(x, np.float32)
    graph_ids = np.asarray(graph_ids, np.int64)
    idx_c, slot_cols, CAu, CBu = _prep_edges(src, dst)
    nc = _build_program(CAu, CBu)
    in_maps = _make_in_maps(
        x, graph_ids,
        [np.asarray(W0), np.asarray(W1), np.asarray(W2)],
        [np.asarray(b0, np.float32), np.asarray(b1, np.float32),
         np.asarray(b2, np.float32)],
        Wc, bc, idx_c, slot_cols)
    res = bass_utils.run_bass_kernel_spmd(nc, in_maps, core_ids=list(range(NCORES)))
    return np.asarray(res.results[0]["out"], np.float32)


# revision 24
# speedup vs baseline: 9.8213x; 1.1597x over previous
"""TAGConv GNN classifier on 8 Trainium2 NeuronCores.

Sharding: nodes split into 8 contiguous slices (6250/core, padded to 6272);
edges live on the core that owns their dst. Each hop: every core gathers
src rows from a replicated norm-prescaled bf16 node table in HBM
(dma_gather, int16 indices -> split-table trick), segment-sums them into
its dst slice with one-hot matmuls on TensorE (PSUM accumulation), rescales
by norm, and all-gathers its slice of the next table. Readout partial sums
per graph are all-reduced, then every core computes the (identical) logits.

Per-group work is uniform (chunk counts padded to the max) so each pass is
a single hardware For_i loop over the 49 dst groups -> ~40x fewer
instructions than full unrolling, which cuts per-call program
serialize/load overhead. Inputs are uploaded compactly (bf16 features and
weights, de-replicated int16 gather indices, uint8 slots/graph-ids) to cut
host->device transfer bytes.
"""
import os

import numpy as np
import ml_dtypes

import concourse.bass as bass
import concourse.bacc as bacc
import concourse.mybir as mybir
import concourse.tile as tile
from concourse import bass_utils
from concourse.bass import ds

N, E, G = 50000, 800000, 128
F = 128                      # IN_DIM == HID
CLASSES = 10
HOPS, HLAYERS = 2, 2         # 3 TAGConv layers total
NCORES = 8

PER = N // NCORES            # real nodes per core
GRP = (PER + 127) // 128     # dst groups of 128 per core
NPAD = GRP * 128             # padded nodes per core
NT = NCORES * NPAD           # padded total
HALF = NT // 2               # int16-safe split of the node table

FP = mybir.dt.float32
BF = mybir.dt.bfloat16
I16 = mybir.dt.int16
U8 = mybir.dt.uint8
NPBF = ml_dtypes.bfloat16


def _prep_edges(src, dst):
    """Per-core gather-index + slot tables with uniform chunks per group."""
    src = np.asarray(src).astype(np.int64)
    dst = np.asarray(dst).astype(np.int64)
    core = dst // PER
    local = dst - core * PER
    grp = local // 128
    slot = local % 128
    ps = (src // PER) * NPAD + (src % PER)          # padded global src id
    half = (ps >= HALF).astype(np.int64)
    idxv = ps - half * HALF                          # int16-safe index

    key = (core * GRP + grp) * 2 + half
    order = np.argsort(key, kind="stable")
    cnt = np.bincount(key, minlength=NCORES * GRP * 2).reshape(NCORES, GRP, 2)
    CAu = max(1, -(-int(cnt[:, :, 0].max()) // 128))
    CBu = max(1, -(-int(cnt[:, :, 1].max()) // 128))
    CH = CAu + CBu
    TOT = GRP * CH * 128

    idx16 = np.zeros((NCORES, TOT), np.int16)
    slotu = np.full((NCORES, TOT), 255, np.uint8)
    sidx = idxv[order]
    sslot = slot[order].astype(np.uint8)
    starts = np.concatenate([[0], np.cumsum(cnt.reshape(-1))]).astype(int)
    for c in range(NCORES):
        for g in range(GRP):
            base = g * CH * 128
            for h, off in ((0, base), (1, base + CAu * 128)):
                k = (c * GRP + g) * 2 + h
                n = int(cnt[c, g, h])
                s0 = starts[k]
                idx16[c, off : off + n] = sidx[s0 : s0 + n]
                slotu[c, off : off + n] = sslot[s0 : s0 + n]

    idx_c = np.ascontiguousarray(idx16.reshape(NCORES, -1, 16).transpose(0, 2, 1))
    slot_cols = np.ascontiguousarray(
        slotu.reshape(NCORES, GRP * CH, 128).transpose(0, 2, 1))
    return idx_c, slot_cols, CAu, CBu


def _build_program(CAu, CBu):
    STAGE = os.environ.get("KSTAGE", "full")
    ORDER = ["deg", "t0", "ag0", "hop1", "aghop", "hop2", "layer0", "full"]
    LVL = ORDER.index(STAGE)
    CH = CAu + CBu
    NCH = GRP * CH
    TOT = NCH * 128
    W16 = TOT // 16
    nc = bacc.Bacc("TRN2", target_bir_lowering=False, debug=False, num_devices=NCORES)
    RG = [list(range(NCORES))]

    I8 = mybir.dt.int8
    # misc layout (fp32 columns): [0:3]=b_cols, [3:13]=wc, [13:23]=bc_rep,
    # [23:23+GRP]=x row scales (slot-major), [23+GRP:23+2*GRP]=gslot as fp32.
    MC_B, MC_WC, MC_BC = 0, 3, 3 + CLASSES
    MC_XS = MC_BC + CLASSES
    MC_GS = MC_XS + GRP
    MCOLS = MC_GS + GRP
    # single packed i8 input: x (slot-major blocks), slots, weights, misc
    X_OFF, SLOT_OFF, W_OFF, MISC_OFF, PCOLS = _pack_offsets(NCH)
    pack_d = nc.dram_tensor("pack", [128, PCOLS], I8, kind="ExternalInput")
    idx_d = nc.dram_tensor("idx_c", [16, W16], I16, kind="ExternalInput")
    out_d = nc.dram_tensor("out", [G, CLASSES], FP, kind="ExternalOutput")

    with tile.TileContext(nc) as tc:
        with (
            tc.tile_pool(name="const", bufs=1) as cp,
            tc.tile_pool(name="work", bufs=2) as wp,
            tc.tile_pool(name="psmm", bufs=3, space="PSUM") as pmm,
            tc.tile_pool(name="pstr", bufs=2, space="PSUM") as ptr,
            tc.tile_pool(name="psro", bufs=2, space="PSUM") as pro,
            tc.tile_pool(name="dram", bufs=1, space="DRAM") as dp,
        ):
            # ---- persistent tiles ----
            idx_t = cp.tile([128, W16], I16)
            slot8_t = cp.tile([128, NCH], I8)
            slot_t = cp.tile([128, NCH], BF)
            misc_t = cp.tile([128, MCOLS], FP)
            gslot_t = cp.tile([128, GRP], BF)
            xsb_t = cp.tile([128, GRP], BF)
            iota_b = cp.tile([128, 128], BF)
            iota_f = cp.tile([128, 128], FP)
            ident_b = cp.tile([128, 128], BF)
            ident_f = cp.tile([128, 128], FP)
            ones_b = cp.tile([128, 1], BF)
            normc_t = cp.tile([128, GRP], FP)
            normb_t = cp.tile([128, GRP], BF)
            w_t = [cp.tile([128, HOPS + 1, F], BF, name=f"w{l}_t", tag=f"w{l}")
                   for l in range(HLAYERS + 1)]
            f0T = cp.tile([128, GRP * 128], BF)   # feat-major [f, i] per group
            f1T = cp.tile([128, GRP * 128], BF)
            f2T = cp.tile([128, GRP * 128], BF)
            roacc_t = cp.tile([128, F + 1], FP)
            ro2_t = cp.tile([128, F + 1], FP)
            cnt_t = cp.tile([128, 1], FP)
            rcp_t = cp.tile([128, 1], FP)
            hg_t = cp.tile([128, F], FP)
            hgT_t = cp.tile([F, 128], FP)
            logit_t = cp.tile([128, CLASSES], FP)

            T_in = dp.tile([NT, F], BF)
            T_hop = dp.tile([NT, F], BF)
            ag_in = dp.tile([NPAD, F], BF)
            ar_in = dp.tile([128, F + 1], FP)
            ar_out = dp.tile([128, F + 1], FP)

            # ---- constants ----
            for p in range(8):
                nc.sync.dma_start(idx_t[p * 16 : (p + 1) * 16, :], idx_d[:, :])
            nc.sync.dma_start(slot8_t[:], pack_d[:, SLOT_OFF : SLOT_OFF + NCH])
            nc.sync.dma_start(
                misc_t[:], pack_d[:, MISC_OFF : MISC_OFF + MCOLS * 4].bitcast(FP))
            nc.vector.tensor_copy(slot_t[:], slot8_t[:])
            nc.vector.tensor_copy(gslot_t[:], misc_t[:, MC_GS : MC_GS + GRP])
            nc.vector.tensor_copy(xsb_t[:], misc_t[:, MC_XS : MC_XS + GRP])
            for l in range(HLAYERS + 1):
                for k in range(HOPS + 1):
                    c0 = W_OFF + (l * (HOPS + 1) + k) * F * 2
                    nc.sync.dma_start(w_t[l][:, k, :],
                                      pack_d[:, c0 : c0 + F * 2].bitcast(BF))

            nc.gpsimd.iota(iota_f[:], pattern=[[1, 128]], base=0, channel_multiplier=0,
                           allow_small_or_imprecise_dtypes=True)
            nc.vector.tensor_copy(iota_b[:], iota_f[:])
            icol_t = cp.tile([128, 1], FP)
            nc.gpsimd.iota(icol_t[:], pattern=[[0, 1]], base=0, channel_multiplier=1,
                           allow_small_or_imprecise_dtypes=True)
            nc.vector.tensor_tensor(ident_f[:], icol_t[:].broadcast_to([128, 128]),
                                    iota_f[:], mybir.AluOpType.is_equal)
            nc.vector.tensor_copy(ident_b[:], ident_f[:])
            nc.vector.memset(ones_b[:], 1.0)
            nc.vector.memset(roacc_t[:], 0.0)

            def bail():
                nc.vector.tensor_copy(logit_t[:], iota_f[:, :CLASSES])
                nc.sync.dma_start(out_d[:, :], logit_t[:])

            def onehot(g):
                """[128e, CH, 128j] one-hot tile for group g (one DVE op)."""
                oh = wp.tile([128, CH, 128], BF, name="oh", tag="oh")
                nc.vector.tensor_tensor(
                    oh[:, :, :],
                    slot_t[:, ds(g * CH, CH)].unsqueeze(2).broadcast_to([128, CH, 128]),
                    iota_b[:].unsqueeze(1).broadcast_to([128, CH, 128]),
                    mybir.AluOpType.is_equal,
                )
                return oh

            # ---- degree / norm pass ----
            with tc.For_i(0, GRP, 1) as g:
                oh = onehot(g)
                dps = pmm.tile([128, 128], FP, name="dps", tag="mm")
                for c in range(CH):
                    nc.tensor.matmul(dps[:, 0:1], oh[:, c, :], ones_b[:],
                                     start=(c == 0), stop=(c == CH - 1))
                dmx = wp.tile([128, 1], FP, name="dmx", tag="dmx")
                nc.vector.tensor_scalar_max(dmx[:], dps[:, 0:1], 1.0)
                drc = wp.tile([128, 1], FP, name="drc", tag="drc")
                nc.vector.reciprocal(drc[:], dmx[:])
                nc.scalar.activation(normc_t[:, ds(g, 1)], drc[:],
                                     mybir.ActivationFunctionType.Sqrt)
            nc.vector.tensor_copy(normb_t[:], normc_t[:])
            STOP = LVL <= ORDER.index("deg")
            if STOP:
                bail()

            # ---- T0 = x * norm ; f0T = x^T ----
            if not STOP:
                with tc.For_i(0, GRP, 1) as g:
                    x8 = wp.tile([128, F], I8, name="x8", tag="x8")
                    nc.sync.dma_start(x8[:], pack_d[:, ds(g * F, F)])
                    xb = wp.tile([128, F], BF, name="xb", tag="xb")
                    nc.vector.tensor_copy(xb[:], x8[:])
                    xt = wp.tile([128, F], BF, name="xt", tag="xt")
                    nc.vector.tensor_tensor(
                        xt[:], xb[:], xsb_t[:, ds(g, 1)].broadcast_to([128, F]),
                        mybir.AluOpType.mult)
                    t0 = wp.tile([128, F], BF, name="t0", tag="tn")
                    nc.vector.tensor_tensor(
                        t0[:], xt[:], normb_t[:, ds(g, 1)].broadcast_to([128, F]),
                        mybir.AluOpType.mult)
                    nc.sync.dma_start(ag_in[ds(g * 128, 128), :], t0[:])
                    pt = ptr.tile([128, 128], BF, name="pt", tag="tr")
                    nc.tensor.transpose(pt[:], xt[:], ident_b[:])
                    nc.vector.tensor_copy(f0T[:, ds(g * 128, 128)], pt[:])
            if not STOP and LVL <= ORDER.index("t0"):
                bail()
                STOP = True
            if not STOP:
                nc.gpsimd.collective_compute(
                    "AllGather", mybir.AluOpType.bypass, replica_groups=RG,
                    ins=[ag_in.opt()], outs=[T_in.opt()])
            if not STOP and LVL <= ORDER.index("ag0"):
                bail()
                STOP = True

            def hop(src_tbl, fT, make_table):
                """One SpMM hop: gather -> one-hot segsum -> scale; optionally
                also emit next scaled table slice into ag_in."""
                with tc.For_i(0, GRP, 1) as g:
                    vb = wp.tile([128, CH, 128], BF, name="vb", tag="vb")
                    nc.gpsimd.dma_gather(
                        vb[:, 0:CAu, :], src_tbl[:, :],
                        idx_t[:, ds(g * CH * 8, CAu * 8)],
                        CAu * 128, CAu * 128, F, single_packet=False)
                    nc.gpsimd.dma_gather(
                        vb[:, CAu:CH, :], src_tbl[HALF:, :],
                        idx_t[:, ds(g * CH * 8 + CAu * 8, CBu * 8)],
                        CBu * 128, CBu * 128, F, single_packet=False)
                    oh = onehot(g)
                    ps = pmm.tile([128, 128], FP, name="ps", tag="mm")
                    for c in range(CH):
                        nc.tensor.matmul(ps[:], oh[:, c, :], vb[:, c, :],
                                         start=(c == 0), stop=(c == CH - 1))
                    fn = wp.tile([128, F], BF, name="fn", tag="fn")
                    nc.vector.tensor_tensor(
                        fn[:], ps[:], normc_t[:, ds(g, 1)].broadcast_to([128, F]),
                        mybir.AluOpType.mult)
                    if make_table:
                        tn = wp.tile([128, F], BF, name="tn", tag="tn")
                        nc.vector.tensor_tensor(
                            tn[:], fn[:], normb_t[:, ds(g, 1)].broadcast_to([128, F]),
                            mybir.AluOpType.mult)
                        nc.sync.dma_start(ag_in[ds(g * 128, 128), :], tn[:])
                    pt = ptr.tile([128, 128], BF, name="pt2", tag="tr")
                    nc.tensor.transpose(pt[:], fn[:], ident_b[:])
                    nc.vector.tensor_copy(fT[:, ds(g * 128, 128)], pt[:])

            for l in range(HLAYERS + 1) if not STOP else []:
                hop(T_in, f1T, make_table=True)
                if l == 0 and LVL <= ORDER.index("hop1"):
                    bail()
                    STOP = True
                    break
                nc.gpsimd.collective_compute(
                    "AllGather", mybir.AluOpType.bypass, replica_groups=RG,
                    ins=[ag_in.opt()], outs=[T_hop.opt()])
                if l == 0 and LVL <= ORDER.index("aghop"):
                    bail()
                    STOP = True
                    break
                hop(T_hop, f2T, make_table=False)
                if l == 0 and LVL <= ORDER.index("hop2"):
                    bail()
                    STOP = True
                    break
                fTs = [f0T, f1T, f2T]
                with tc.For_i(0, GRP, 1) as g:
                    ph = pmm.tile([128, 128], FP, name="ph", tag="mm")
                    for k in range(HOPS + 1):
                        nc.tensor.matmul(ph[:], w_t[l][:, k, :],
                                         fTs[k][:, ds(g * 128, 128)],
                                         start=(k == 0), stop=(k == HOPS))
                    act = wp.tile([128, 128], BF, name="act", tag="act")
                    nc.scalar.activation(act[:], ph[:],
                                         mybir.ActivationFunctionType.Relu,
                                         bias=misc_t[:, MC_B + l : MC_B + l + 1])
                    nc.vector.tensor_copy(f0T[:, ds(g * 128, 128)], act[:])
                    pt = ptr.tile([128, 128], BF, name="pt3", tag="tr")
                    nc.tensor.transpose(pt[:], act[:], ident_b[:])
                    if l < HLAYERS:
                        tn = wp.tile([128, F], BF, name="tn2", tag="tn")
                        nc.vector.tensor_tensor(
                            tn[:], pt[:], normb_t[:, ds(g, 1)].broadcast_to([128, F]),
                            mybir.AluOpType.mult)
                        nc.sync.dma_start(ag_in[ds(g * 128, 128), :], tn[:])
                    else:
                        rr = wp.tile([128, F + 1], BF, name="rr", tag="rr")
                        nc.vector.tensor_copy(rr[:, 0:F], pt[:])
                        nc.vector.tensor_copy(rr[:, F : F + 1], ones_b[:])
                        og = wp.tile([128, 128], BF, name="og", tag="og")
                        nc.vector.tensor_tensor(
                            og[:], gslot_t[:, ds(g, 1)].broadcast_to([128, 128]),
                            iota_b[:], mybir.AluOpType.is_equal)
                        pr = pro.tile([128, F + 1], FP, name="pr", tag="ro")
                        nc.tensor.matmul(pr[:], og[:], rr[:], start=True, stop=True)
                        nc.vector.tensor_tensor(roacc_t[:], roacc_t[:], pr[:],
                                                mybir.AluOpType.add)
                if l < HLAYERS:
                    nc.gpsimd.collective_compute(
                        "AllGather", mybir.AluOpType.bypass, replica_groups=RG,
                        ins=[ag_in.opt()], outs=[T_in.opt()])
                if l == 0 and LVL <= ORDER.index("layer0"):
                    bail()
                    STOP = True
                    break

            # ---- readout: all-reduce partial sums, mean, classify ----
            if not STOP:
                nc.sync.dma_start(ar_in[:, :], roacc_t[:])
                nc.gpsimd.collective_compute(
                    "AllReduce", mybir.AluOpType.add, replica_groups=RG,
                    ins=[ar_in.opt()], outs=[ar_out.opt()])
                nc.sync.dma_start(ro2_t[:], ar_out[:, :])
                nc.vector.tensor_scalar_max(cnt_t[:], ro2_t[:, F : F + 1], 1.0)
                nc.vector.reciprocal(rcp_t[:], cnt_t[:])
                nc.vector.tensor_tensor(hg_t[:], ro2_t[:, 0:F],
                                        rcp_t[:].broadcast_to([128, F]),
                                        mybir.AluOpType.mult)
                ptf = ptr.tile([128, 128], FP, name="ptf", tag="tr")
                nc.tensor.transpose(ptf[:], hg_t[:], ident_f[:])
                nc.vector.tensor_copy(hgT_t[:], ptf[:])
                plog = pro.tile([128, F + 1], FP, name="plog", tag="ro")
                nc.tensor.matmul(plog[:, 0:CLASSES], hgT_t[:],
                                 misc_t[:, MC_WC : MC_WC + CLASSES],
                                 start=True, stop=True)
                nc.vector.tensor_tensor(logit_t[:], plog[:, 0:CLASSES],
                                        misc_t[:, MC_BC : MC_BC + CLASSES],
                                        mybir.AluOpType.add)
                nc.sync.dma_start(out_d[:, :], logit_t[:])

    nc.finalize()
    return nc


def _make_in_maps(x, graph_ids, Ws, bs, Wc, bc, idx_c, slot_cols):
    b_cols = np.stack(bs, axis=1).astype(np.float32)            # [128, 3]
    bc_rep = np.tile(np.asarray(bc, np.float32)[None, :], (128, 1))
    w_bf = np.concatenate([np.asarray(w, np.float32) for w in Ws], axis=0).astype(NPBF)
    wc_f = np.asarray(Wc, np.float32)
    # per-node int8 quantization of x
    x_full8 = np.zeros((N, F), np.int8)
    xs_full = np.ones(N, np.float32)
    amax = np.abs(x).max(axis=1)
    nz = amax > 0
    xs_full[nz] = amax[nz] / 127.0
    x_full8 = np.clip(np.round(x / xs_full[:, None]), -127, 127).astype(np.int8)
    # weights packed slot-major: [128, 9*256] bytes
    w_pack = np.ascontiguousarray(
        w_bf.reshape(3 * (HOPS + 1), 128, F).transpose(1, 0, 2)
    ).view(np.int8).reshape(128, -1)
    in_maps = []
    for c in range(NCORES):
        x_loc = np.zeros((NPAD, F), np.int8)
        x_loc[:PER] = x_full8[c * PER : (c + 1) * PER]
        x_pack = np.ascontiguousarray(
            x_loc.reshape(GRP, 128, F).transpose(1, 0, 2)).reshape(128, GRP * F)
        xs = np.ones(NPAD, np.float32)
        xs[:PER] = xs_full[c * PER : (c + 1) * PER]
        gsl = np.full(NPAD, 255.0, np.float32)
        gsl[:PER] = graph_ids[c * PER : (c + 1) * PER].astype(np.float32)
        misc = np.concatenate([
            b_cols, wc_f, bc_rep,
            np.ascontiguousarray(xs.reshape(GRP, 128).T),
            np.ascontiguousarray(gsl.reshape(GRP, 128).T),
        ], axis=1).astype(np.float32)
        parts = [x_pack, slot_cols[c].view(np.int8), w_pack,
                 np.ascontiguousarray(misc).view(np.int8)]
        X_OFF, SLOT_OFF, W_OFF, MISC_OFF, PCOLS = _pack_offsets(slot_cols.shape[2])
        pack = np.zeros((128, PCOLS), np.int8)
        for p, o in zip(parts, (X_OFF, SLOT_OFF, W_OFF, MISC_OFF)):
            pack[:, o : o + p.shape[1]] = p
        in_maps.append(dict(pack=pack, idx_c=idx_c[c]))
    return in_maps


def _pack_offsets(NCH):
    MCOLS = 3 + CLASSES + CLASSES + GRP + GRP
    SLOT_OFF = GRP * F
    W_OFF = -(-(SLOT_OFF + NCH) // 4) * 4
    MISC_OFF = -(-(W_OFF + (HLAYERS + 1) * (HOPS + 1) * F * 2) // 4) * 4
    PCOLS = MISC_OFF + MCOLS * 4
    return 0, SLOT_OFF, W_OFF, MISC_OFF, PCOLS


def kernel(x, src, dst, graph_ids, W0, b0, W1, b1, W2, b2, Wc, bc, **_):
    x = np.asarray(x, np.float32)
    graph_ids = np.asarray(graph_ids, np.int64)
    idx_c, slot_cols, CAu, CBu = _prep_edges(src, dst)
    nc = _build_program(CAu, CBu)
    in_maps = _make_in_maps(
        x, graph_ids,
        [np.asarray(W0), np.asarray(W1), np.asarray(W2)],
        [np.asarray(b0, np.float32), np.asarray(b1, np.float32),
         np.asarray(b2, np.float32)],
        Wc, bc, idx_c, slot_cols)
    res = bass_utils.run_bass_kernel_spmd(nc, in_maps, core_ids=list(range(NCORES)))
    return np.asarray(res.results[0]["out"], np.float32)


# revision 28
# speedup vs baseline: 9.9346x; 1.0115x over previous
"""TAGConv GNN classifier on 8 Trainium2 NeuronCores.

Sharding: nodes split into 8 contiguous slices (6250/core, padded to 6272);
edges live on the core that owns their dst. Each hop: every core gathers
src rows from a replicated norm-prescaled bf16 node table in HBM
(dma_gather, int16 indices -> split-table trick), segment-sums them into
its dst slice with one-hot matmuls on TensorE (PSUM accumulation), rescales
by norm, and all-gathers its slice of the next table. Readout partial sums
per graph are all-reduced, then every core computes the (identical) logits.

Per-group work is uniform (chunk counts padded to the max) so each pass is
a single hardware For_i loop over the 49 dst groups -> ~40x fewer
instructions than full unrolling, which cuts per-call program
serialize/load overhead. Inputs are uploaded compactly (bf16 features and
weights, de-replicated int16 gather indices, uint8 slots/graph-ids) to cut
host->device transfer bytes.
"""
import os

import numpy as np
import ml_dtypes

import concourse.bass as bass
import concourse.bacc as bacc
import concourse.mybir as mybir
import concourse.tile as tile
from concourse import bass_utils
from concourse.bass import ds

N, E, G = 50000, 800000, 128
F = 128                      # IN_DIM == HID
CLASSES = 10
HOPS, HLAYERS = 2, 2         # 3 TAGConv layers total
NCORES = 8

PER = N // NCORES            # real nodes per core
GRP = (PER + 127) // 128     # dst groups of 128 per core
NPAD = GRP * 128             # padded nodes per core
NT = NCORES * NPAD           # padded total
HALF = NT // 2               # int16-safe split of the node table

FP = mybir.dt.float32
BF = mybir.dt.bfloat16
I16 = mybir.dt.int16
U8 = mybir.dt.uint8
NPBF = ml_dtypes.bfloat16


def _prep_edges(src, dst):
    """Per-core gather-index + slot tables with uniform chunks per group."""
    src = np.asarray(src).astype(np.int64)
    dst = np.asarray(dst).astype(np.int64)
    core = dst // PER
    local = dst - core * PER
    grp = local // 128
    slot = local % 128
    ps = (src // PER) * NPAD + (src % PER)          # padded global src id
    half = (ps >= HALF).astype(np.int64)
    idxv = ps - half * HALF                          # int16-safe index

    key = (core * GRP + grp) * 2 + half
    order = np.argsort(key, kind="stable")
    cnt = np.bincount(key, minlength=NCORES * GRP * 2).reshape(NCORES, GRP, 2)
    CAu = max(1, -(-int(cnt[:, :, 0].max()) // 128))
    CBu = max(1, -(-int(cnt[:, :, 1].max()) // 128))
    CH = CAu + CBu
    TOT = GRP * CH * 128

    idx16 = np.zeros((NCORES, TOT), np.int16)
    slotu = np.full((NCORES, TOT), 255, np.uint8)
    sidx = idxv[order]
    sslot = slot[order].astype(np.uint8)
    starts = np.concatenate([[0], np.cumsum(cnt.reshape(-1))]).astype(int)
    for c in range(NCORES):
        for g in range(GRP):
            base = g * CH * 128
            for h, off in ((0, base), (1, base + CAu * 128)):
                k = (c * GRP + g) * 2 + h
                n = int(cnt[c, g, h])
                s0 = starts[k]
                idx16[c, off : off + n] = sidx[s0 : s0 + n]
                slotu[c, off : off + n] = sslot[s0 : s0 + n]

    idx_c = np.ascontiguousarray(idx16.reshape(NCORES, -1, 16).transpose(0, 2, 1))
    slot_cols = np.ascontiguousarray(
        slotu.reshape(NCORES, GRP * CH, 128).transpose(0, 2, 1))
    return idx_c, slot_cols, CAu, CBu


def _build_program(CAu, CBu):
    STAGE = os.environ.get("KSTAGE", "full")
    ORDER = ["deg", "t0", "ag0", "hop1", "aghop", "hop2", "layer0", "full"]
    LVL = ORDER.index(STAGE)
    CH = CAu + CBu
    NCH = GRP * CH
    TOT = NCH * 128
    W16 = TOT // 16
    nc = bacc.Bacc("TRN2", target_bir_lowering=False, debug=False, num_devices=NCORES)
    RG = [list(range(NCORES))]

    I8 = mybir.dt.int8
    # misc layout (fp32 columns): [0:3]=b_cols, [3:13]=wc, [13:23]=bc_rep,
    # [23:23+GRP]=x row scales (slot-major), [23+GRP:23+2*GRP]=gslot as fp32.
    MC_B, MC_WC, MC_BC = 0, 3, 3 + CLASSES
    MC_XS = MC_BC + CLASSES
    MC_GS = MC_XS + GRP
    MCOLS = MC_GS + GRP
    # single packed i8 input: x (slot-major blocks), slots, weights, misc, idx
    X_OFF, SLOT_OFF, W_OFF, MISC_OFF, IDX_OFF, PCOLS = _pack_offsets(NCH)
    W128 = W16 // 8
    pack_d = nc.dram_tensor("pack", [128, PCOLS], I8, kind="ExternalInput")
    out_d = nc.dram_tensor("out", [G, CLASSES], FP, kind="ExternalOutput")

    with tile.TileContext(nc) as tc:
        with (
            tc.tile_pool(name="const", bufs=1) as cp,
            tc.tile_pool(name="work", bufs=2) as wp,
            tc.tile_pool(name="psmm", bufs=3, space="PSUM") as pmm,
            tc.tile_pool(name="pstr", bufs=2, space="PSUM") as ptr,
            tc.tile_pool(name="psro", bufs=2, space="PSUM") as pro,
            tc.tile_pool(name="dram", bufs=1, space="DRAM") as dp,
        ):
            # ---- persistent tiles ----
            idx_t = cp.tile([128, W16], I16)
            slot8_t = cp.tile([128, NCH], I8)
            slot_t = cp.tile([128, NCH], BF)
            misc_t = cp.tile([128, MCOLS], FP)
            gslot_t = cp.tile([128, GRP], BF)
            xsb_t = cp.tile([128, GRP], BF)
            iota_b = cp.tile([128, 128], BF)
            iota_f = cp.tile([128, 128], FP)
            ident_b = cp.tile([128, 128], BF)
            ident_f = cp.tile([128, 128], FP)
            ones_b = cp.tile([128, 1], BF)
            normc_t = cp.tile([128, GRP], FP)
            normb_t = cp.tile([128, GRP], BF)
            w_t = [cp.tile([128, HOPS + 1, F], BF, name=f"w{l}_t", tag=f"w{l}")
                   for l in range(HLAYERS + 1)]
            f0T = cp.tile([128, GRP * 128], BF)   # feat-major [f, i] per group
            f1T = cp.tile([128, GRP * 128], BF)
            f2T = cp.tile([128, GRP * 128], BF)
            roacc_t = cp.tile([128, F + 1], FP)
            ro2_t = cp.tile([128, F + 1], FP)
            cnt_t = cp.tile([128, 1], FP)
            rcp_t = cp.tile([128, 1], FP)
            hg_t = cp.tile([128, F], FP)
            hgT_t = cp.tile([F, 128], FP)
            logit_t = cp.tile([128, CLASSES], FP)

            T_in = dp.tile([NT, F], BF)
            T_hop = dp.tile([NT, F], BF)
            ag_in = dp.tile([NPAD, F], BF)
            ar_in = dp.tile([128, F + 1], FP)
            ar_out = dp.tile([128, F + 1], FP)

            # ---- constants ----
            # idx arrives as [128, W128] i16 bytes where row 16a+b holds
            # idx_c[b, a*W128 : (a+1)*W128]; expand to the gather's
            # [128, W16] layout (16-partition wrap replicated 8x).
            for a in range(8):
                for p in range(8):
                    nc.sync.dma_start(
                        idx_t[p * 16 : (p + 1) * 16, a * W128 : (a + 1) * W128],
                        pack_d[16 * a : 16 * a + 16,
                               IDX_OFF : IDX_OFF + W128 * 2].bitcast(I16))
            nc.sync.dma_start(slot8_t[:], pack_d[:, SLOT_OFF : SLOT_OFF + NCH])
            nc.sync.dma_start(
                misc_t[:], pack_d[:, MISC_OFF : MISC_OFF + MCOLS * 4].bitcast(FP))
            nc.vector.tensor_copy(slot_t[:], slot8_t[:])
            nc.vector.tensor_copy(gslot_t[:], misc_t[:, MC_GS : MC_GS + GRP])
            nc.vector.tensor_copy(xsb_t[:], misc_t[:, MC_XS : MC_XS + GRP])
            for l in range(HLAYERS + 1):
                for k in range(HOPS + 1):
                    c0 = W_OFF + (l * (HOPS + 1) + k) * F * 2
                    nc.sync.dma_start(w_t[l][:, k, :],
                                      pack_d[:, c0 : c0 + F * 2].bitcast(BF))

            nc.gpsimd.iota(iota_f[:], pattern=[[1, 128]], base=0, channel_multiplier=0,
                           allow_small_or_imprecise_dtypes=True)
            nc.vector.tensor_copy(iota_b[:], iota_f[:])
            icol_t = cp.tile([128, 1], FP)
            nc.gpsimd.iota(icol_t[:], pattern=[[0, 1]], base=0, channel_multiplier=1,
                           allow_small_or_imprecise_dtypes=True)
            nc.vector.tensor_tensor(ident_f[:], icol_t[:].broadcast_to([128, 128]),
                                    iota_f[:], mybir.AluOpType.is_equal)
            nc.vector.tensor_copy(ident_b[:], ident_f[:])
            nc.vector.memset(ones_b[:], 1.0)
            nc.vector.memset(roacc_t[:], 0.0)

            def bail():
                nc.vector.tensor_copy(logit_t[:], iota_f[:, :CLASSES])
                nc.sync.dma_start(out_d[:, :], logit_t[:])

            def onehot(g):
                """[128e, CH, 128j] one-hot tile for group g (one DVE op)."""
                oh = wp.tile([128, CH, 128], BF, name="oh", tag="oh")
                nc.vector.tensor_tensor(
                    oh[:, :, :],
                    slot_t[:, ds(g * CH, CH)].unsqueeze(2).broadcast_to([128, CH, 128]),
                    iota_b[:].unsqueeze(1).broadcast_to([128, CH, 128]),
                    mybir.AluOpType.is_equal,
                )
                return oh

            # ---- degree / norm pass ----
            with tc.For_i(0, GRP, 1) as g:
                oh = onehot(g)
                dps = pmm.tile([128, 128], FP, name="dps", tag="mm")
                for c in range(CH):
                    nc.tensor.matmul(dps[:, 0:1], oh[:, c, :], ones_b[:],
                                     start=(c == 0), stop=(c == CH - 1))
                dmx = wp.tile([128, 1], FP, name="dmx", tag="dmx")
                nc.vector.tensor_scalar_max(dmx[:], dps[:, 0:1], 1.0)
                drc = wp.tile([128, 1], FP, name="drc", tag="drc")
                nc.vector.reciprocal(drc[:], dmx[:])
                nc.scalar.activation(normc_t[:, ds(g, 1)], drc[:],
                                     mybir.ActivationFunctionType.Sqrt)
            nc.vector.tensor_copy(normb_t[:], normc_t[:])
            STOP = LVL <= ORDER.index("deg")
            if STOP:
                bail()

            # ---- T0 = x * norm ; f0T = x^T ----
            if not STOP:
                with tc.For_i(0, GRP, 1) as g:
                    x8 = wp.tile([128, F], I8, name="x8", tag="x8")
                    nc.sync.dma_start(x8[:], pack_d[:, ds(g * F, F)])
                    xb = wp.tile([128, F], BF, name="xb", tag="xb")
                    nc.vector.tensor_copy(xb[:], x8[:])
                    xt = wp.tile([128, F], BF, name="xt", tag="xt")
                    nc.vector.tensor_tensor(
                        xt[:], xb[:], xsb_t[:, ds(g, 1)].broadcast_to([128, F]),
                        mybir.AluOpType.mult)
                    t0 = wp.tile([128, F], BF, name="t0", tag="tn")
                    nc.vector.tensor_tensor(
                        t0[:], xt[:], normb_t[:, ds(g, 1)].broadcast_to([128, F]),
                        mybir.AluOpType.mult)
                    nc.sync.dma_start(ag_in[ds(g * 128, 128), :], t0[:])
                    pt = ptr.tile([128, 128], BF, name="pt", tag="tr")
                    nc.tensor.transpose(pt[:], xt[:], ident_b[:])
                    nc.vector.tensor_copy(f0T[:, ds(g * 128, 128)], pt[:])
            if not STOP and LVL <= ORDER.index("t0"):
                bail()
                STOP = True
            if not STOP:
                nc.gpsimd.collective_compute(
                    "AllGather", mybir.AluOpType.bypass, replica_groups=RG,
                    ins=[ag_in.opt()], outs=[T_in.opt()])
            if not STOP and LVL <= ORDER.index("ag0"):
                bail()
                STOP = True

            def hop(src_tbl, fT, make_table):
                """One SpMM hop: gather -> one-hot segsum -> scale; optionally
                also emit next scaled table slice into ag_in."""
                with tc.For_i(0, GRP, 1) as g:
                    vb = wp.tile([128, CH, 128], BF, name="vb", tag="vb")
                    nc.gpsimd.dma_gather(
                        vb[:, 0:CAu, :], src_tbl[:, :],
                        idx_t[:, ds(g * CH * 8, CAu * 8)],
                        CAu * 128, CAu * 128, F, single_packet=False)
                    nc.gpsimd.dma_gather(
                        vb[:, CAu:CH, :], src_tbl[HALF:, :],
                        idx_t[:, ds(g * CH * 8 + CAu * 8, CBu * 8)],
                        CBu * 128, CBu * 128, F, single_packet=False)
                    oh = onehot(g)
                    ps = pmm.tile([128, 128], FP, name="ps", tag="mm")
                    for c in range(CH):
                        nc.tensor.matmul(ps[:], oh[:, c, :], vb[:, c, :],
                                         start=(c == 0), stop=(c == CH - 1))
                    fn = wp.tile([128, F], BF, name="fn", tag="fn")
                    nc.vector.tensor_tensor(
                        fn[:], ps[:], normc_t[:, ds(g, 1)].broadcast_to([128, F]),
                        mybir.AluOpType.mult)
                    if make_table:
                        tn = wp.tile([128, F], BF, name="tn", tag="tn")
                        nc.vector.tensor_tensor(
                            tn[:], fn[:], normb_t[:, ds(g, 1)].broadcast_to([128, F]),
                            mybir.AluOpType.mult)
                        nc.sync.dma_start(ag_in[ds(g * 128, 128), :], tn[:])
                    pt = ptr.tile([128, 128], BF, name="pt2", tag="tr")
                    nc.tensor.transpose(pt[:], fn[:], ident_b[:])
                    nc.vector.tensor_copy(fT[:, ds(g * 128, 128)], pt[:])

            for l in range(HLAYERS + 1) if not STOP else []:
                hop(T_in, f1T, make_table=True)
                if l == 0 and LVL <= ORDER.index("hop1"):
                    bail()
                    STOP = True
                    break
                nc.gpsimd.collective_compute(
                    "AllGather", mybir.AluOpType.bypass, replica_groups=RG,
                    ins=[ag_in.opt()], outs=[T_hop.opt()])
                if l == 0 and LVL <= ORDER.index("aghop"):
                    bail()
                    STOP = True
                    break
                hop(T_hop, f2T, make_table=False)
                if l == 0 and LVL <= ORDER.index("hop2"):
                    bail()
                    STOP = True
                    break
                fTs = [f0T, f1T, f2T]
                with tc.For_i(0, GRP, 1) as g:
                    ph = pmm.tile([128, 128], FP, name="ph", tag="mm")
                    for k in range(HOPS + 1):
                        nc.tensor.matmul(ph[:], w_t[l][:, k, :],
                                         fTs[k][:, ds(g * 128, 128)],
                                         start=(k == 0), stop=(k == HOPS))
                    act = wp.tile([128, 128], BF, name="act", tag="act")
                    nc.scalar.activation(act[:], ph[:],
                                         mybir.ActivationFunctionType.Relu,
                                         bias=misc_t[:, MC_B + l : MC_B + l + 1])
                    nc.vector.tensor_copy(f0T[:, ds(g * 128, 128)], act[:])
                    pt = ptr.tile([128, 128], BF, name="pt3", tag="tr")
                    nc.tensor.transpose(pt[:], act[:], ident_b[:])
                    if l < HLAYERS:
                        tn = wp.tile([128, F], BF, name="tn2", tag="tn")
                        nc.vector.tensor_tensor(
                            tn[:], pt[:], normb_t[:, ds(g, 1)].broadcast_to([128, F]),
                            mybir.AluOpType.mult)
                        nc.sync.dma_start(ag_in[ds(g * 128, 128), :], tn[:])
                    else:
                        rr = wp.tile([128, F + 1], BF, name="rr", tag="rr")
                        nc.vector.tensor_copy(rr[:, 0:F], pt[:])
                        nc.vector.tensor_copy(rr[:, F : F + 1], ones_b[:])
                        og = wp.tile([128, 128], BF, name="og", tag="og")
                        nc.vector.tensor_tensor(
                            og[:], gslot_t[:, ds(g, 1)].broadcast_to([128, 128]),
                            iota_b[:], mybir.AluOpType.is_equal)
                        pr = pro.tile([128, F + 1], FP, name="pr", tag="ro")
                        nc.tensor.matmul(pr[:], og[:], rr[:], start=True, stop=True)
                        nc.vector.tensor_tensor(roacc_t[:], roacc_t[:], pr[:],
                                                mybir.AluOpType.add)
                if l < HLAYERS:
                    nc.gpsimd.collective_compute(
                        "AllGather", mybir.AluOpType.bypass, replica_groups=RG,
                        ins=[ag_in.opt()], outs=[T_in.opt()])
                if l == 0 and LVL <= ORDER.index("layer0"):
                    bail()
                    STOP = True
                    break

            # ---- readout: all-reduce partial sums, mean, classify ----
            if not STOP:
                nc.sync.dma_start(ar_in[:, :], roacc_t[:])
                nc.gpsimd.collective_compute(
                    "AllReduce", mybir.AluOpType.add, replica_groups=RG,
                    ins=[ar_in.opt()], outs=[ar_out.opt()])
                nc.sync.dma_start(ro2_t[:], ar_out[:, :])
                nc.vector.tensor_scalar_max(cnt_t[:], ro2_t[:, F : F + 1], 1.0)
                nc.vector.reciprocal(rcp_t[:], cnt_t[:])
                nc.vector.tensor_tensor(hg_t[:], ro2_t[:, 0:F],
                                        rcp_t[:].broadcast_to([128, F]),
                                        mybir.AluOpType.mult)
                ptf = ptr.tile([128, 128], FP, name="ptf", tag="tr")
                nc.tensor.transpose(ptf[:], hg_t[:], ident_f[:])
                nc.vector.tensor_copy(hgT_t[:], ptf[:])
                plog = pro.tile([128, F + 1], FP, name="plog", tag="ro")
                nc.tensor.matmul(plog[:, 0:CLASSES], hgT_t[:],
                                 misc_t[:, MC_WC : MC_WC + CLASSES],
                                 start=True, stop=True)
                nc.vector.tensor_tensor(logit_t[:], plog[:, 0:CLASSES],
                                        misc_t[:, MC_BC : MC_BC + CLASSES],
                                        mybir.AluOpType.add)
                nc.sync.dma_start(out_d[:, :], logit_t[:])

    nc.finalize()
    return nc


def _make_in_maps(x, graph_ids, Ws, bs, Wc, bc, idx_c, slot_cols):
    b_cols = np.stack(bs, axis=1).astype(np.float32)            # [128, 3]
    bc_rep = np.tile(np.asarray(bc, np.float32)[None, :], (128, 1))
    w_bf = np.concatenate([np.asarray(w, np.float32) for w in Ws], axis=0).astype(NPBF)
    wc_f = np.asarray(Wc, np.float32)
    # per-node int8 quantization of x
    x_full8 = np.zeros((N, F), np.int8)
    xs_full = np.ones(N, np.float32)
    amax = np.abs(x).max(axis=1)
    nz = amax > 0
    xs_full[nz] = amax[nz] / 127.0
    x_full8 = np.clip(np.round(x / xs_full[:, None]), -127, 127).astype(np.int8)
    # weights packed slot-major: [128, 9*256] bytes
    w_pack = np.ascontiguousarray(
        w_bf.reshape(3 * (HOPS + 1), 128, F).transpose(1, 0, 2)
    ).view(np.int8).reshape(128, -1)
    in_maps = []
    for c in range(NCORES):
        x_loc = np.zeros((NPAD, F), np.int8)
        x_loc[:PER] = x_full8[c * PER : (c + 1) * PER]
        x_pack = np.ascontiguousarray(
            x_loc.reshape(GRP, 128, F).transpose(1, 0, 2)).reshape(128, GRP * F)
        xs = np.ones(NPAD, np.float32)
        xs[:PER] = xs_full[c * PER : (c + 1) * PER]
        gsl = np.full(NPAD, 255.0, np.float32)
        gsl[:PER] = graph_ids[c * PER : (c + 1) * PER].astype(np.float32)
        misc = np.concatenate([
            b_cols, wc_f, bc_rep,
            np.ascontiguousarray(xs.reshape(GRP, 128).T),
            np.ascontiguousarray(gsl.reshape(GRP, 128).T),
        ], axis=1).astype(np.float32)
        W16 = idx_c.shape[2]
        idx_pack = np.ascontiguousarray(
            idx_c[c].reshape(16, 8, W16 // 8).transpose(1, 0, 2)
        ).reshape(128, W16 // 8).view(np.int8)
        parts = [x_pack, slot_cols[c].view(np.int8), w_pack,
                 np.ascontiguousarray(misc).view(np.int8), idx_pack]
        X_OFF, SLOT_OFF, W_OFF, MISC_OFF, IDX_OFF, PCOLS = _pack_offsets(
            slot_cols.shape[2])
        pack = np.zeros((128, PCOLS), np.int8)
        for p, o in zip(parts, (X_OFF, SLOT_OFF, W_OFF, MISC_OFF, IDX_OFF)):
            pack[:, o : o + p.shape[1]] = p
        in_maps.append(dict(pack=pack))
    return in_maps


def _pack_offsets(NCH):
    MCOLS = 3 + CLASSES + CLASSES + GRP + GRP
    W16 = NCH * 128 // 16
    SLOT_OFF = GRP * F
    W_OFF = -(-(SLOT_OFF + NCH) // 4) * 4
    MISC_OFF = -(-(W_OFF + (HLAYERS + 1) * (HOPS + 1) * F * 2) // 4) * 4
    IDX_OFF = MISC_OFF + MCOLS * 4
    PCOLS = IDX_OFF + (W16 // 8) * 2
    return 0, SLOT_OFF, W_OFF, MISC_OFF, IDX_OFF, PCOLS


def kernel(x, src, dst, graph_ids, W0, b0, W1, b1, W2, b2, Wc, bc, **_):
    x = np.asarray(x, np.float32)
    graph_ids = np.asarray(graph_ids, np.int64)
    idx_c, slot_cols, CAu, CBu = _prep_edges(src, dst)
    nc = _build_program(CAu, CBu)
    in_maps = _make_in_maps(
        x, graph_ids,
        [np.asarray(W0), np.asarray(W1), np.asarray(W2)],
        [np.asarray(b0, np.float32), np.asarray(b1, np.float32),
         np.asarray(b2, np.float32)],
        Wc, bc, idx_c, slot_cols)
    res = bass_utils.run_bass_kernel_spmd(nc, in_maps, core_ids=list(range(NCORES)))
    return np.asarray(res.results[0]["out"], np.float32)


# revision 29
# speedup vs baseline: 15.1083x; 1.5208x over previous
"""TAGConv GNN classifier on 8 Trainium2 NeuronCores.

Sharding: nodes split into 8 contiguous slices (6250/core, padded to 6272);
edges live on the core that owns their dst. Each hop: every core gathers
src rows from a replicated norm-prescaled bf16 node table in HBM
(dma_gather, int16 indices -> split-table trick), segment-sums them into
its dst slice with one-hot matmuls on TensorE (PSUM accumulation), rescales
by norm, and all-gathers its slice of the next table. Readout partial sums
per graph are all-reduced, then every core computes the (identical) logits.

Per-group work is uniform (chunk counts padded to the max) so each pass is
a single hardware For_i loop over the 49 dst groups -> ~40x fewer
instructions than full unrolling, which cuts per-call program
serialize/load overhead. Inputs are uploaded compactly (bf16 features and
weights, de-replicated int16 gather indices, uint8 slots/graph-ids) to cut
host->device transfer bytes.
"""
import os
import tempfile

import numpy as np
import ml_dtypes

import jax

# Persistent XLA compilation cache: run_bass_kernel_spmd builds a fresh jit
# per call, so without this every call re-runs the PJRT compile (~130ms via
# the axon tunnel). With it, repeat compiles deserialize from disk (~8ms).
try:
    jax.config.update(
        "jax_compilation_cache_dir",
        os.path.join(tempfile.gettempdir(), "jax_comp_cache"))
    jax.config.update("jax_persistent_cache_min_entry_size_bytes", 0)
    jax.config.update("jax_persistent_cache_min_compile_time_secs", 0.0)
except Exception:
    pass

import concourse.bass as bass
import concourse.bacc as bacc
import concourse.mybir as mybir
import concourse.tile as tile
from concourse import bass_utils
from concourse.bass import ds

N, E, G = 50000, 800000, 128
F = 128                      # IN_DIM == HID
CLASSES = 10
HOPS, HLAYERS = 2, 2         # 3 TAGConv layers total
NCORES = 8

PER = N // NCORES            # real nodes per core
GRP = (PER + 127) // 128     # dst groups of 128 per core
NPAD = GRP * 128             # padded nodes per core
NT = NCORES * NPAD           # padded total
HALF = NT // 2               # int16-safe split of the node table

FP = mybir.dt.float32
BF = mybir.dt.bfloat16
I16 = mybir.dt.int16
U8 = mybir.dt.uint8
NPBF = ml_dtypes.bfloat16


def _prep_edges(src, dst):
    """Per-core gather-index + slot tables with uniform chunks per group."""
    src = np.asarray(src).astype(np.int64)
    dst = np.asarray(dst).astype(np.int64)
    core = dst // PER
    local = dst - core * PER
    grp = local // 128
    slot = local % 128
    ps = (src // PER) * NPAD + (src % PER)          # padded global src id
    half = (ps >= HALF).astype(np.int64)
    idxv = ps - half * HALF                          # int16-safe index

    key = (core * GRP + grp) * 2 + half
    order = np.argsort(key, kind="stable")
    cnt = np.bincount(key, minlength=NCORES * GRP * 2).reshape(NCORES, GRP, 2)
    CAu = max(1, -(-int(cnt[:, :, 0].max()) // 128))
    CBu = max(1, -(-int(cnt[:, :, 1].max()) // 128))
    CH = CAu + CBu
    TOT = GRP * CH * 128

    idx16 = np.zeros((NCORES, TOT), np.int16)
    slotu = np.full((NCORES, TOT), 255, np.uint8)
    sidx = idxv[order]
    sslot = slot[order].astype(np.uint8)
    starts = np.concatenate([[0], np.cumsum(cnt.reshape(-1))]).astype(int)
    for c in range(NCORES):
        for g in range(GRP):
            base = g * CH * 128
            for h, off in ((0, base), (1, base + CAu * 128)):
                k = (c * GRP + g) * 2 + h
                n = int(cnt[c, g, h])
                s0 = starts[k]
                idx16[c, off : off + n] = sidx[s0 : s0 + n]
                slotu[c, off : off + n] = sslot[s0 : s0 + n]

    idx_c = np.ascontiguousarray(idx16.reshape(NCORES, -1, 16).transpose(0, 2, 1))
    slot_cols = np.ascontiguousarray(
        slotu.reshape(NCORES, GRP * CH, 128).transpose(0, 2, 1))
    return idx_c, slot_cols, CAu, CBu


def _build_program(CAu, CBu):
    STAGE = os.environ.get("KSTAGE", "full")
    ORDER = ["deg", "t0", "ag0", "hop1", "aghop", "hop2", "layer0", "full"]
    LVL = ORDER.index(STAGE)
    CH = CAu + CBu
    NCH = GRP * CH
    TOT = NCH * 128
    W16 = TOT // 16
    nc = bacc.Bacc("TRN2", target_bir_lowering=False, debug=False, num_devices=NCORES)
    RG = [list(range(NCORES))]

    I8 = mybir.dt.int8
    # misc layout (fp32 columns): [0:3]=b_cols, [3:13]=wc, [13:23]=bc_rep,
    # [23:23+GRP]=x row scales (slot-major), [23+GRP:23+2*GRP]=gslot as fp32.
    MC_B, MC_WC, MC_BC = 0, 3, 3 + CLASSES
    MC_XS = MC_BC + CLASSES
    MC_GS = MC_XS + GRP
    MCOLS = MC_GS + GRP
    # single packed i8 input: x (slot-major blocks), slots, weights, misc, idx
    X_OFF, SLOT_OFF, W_OFF, MISC_OFF, IDX_OFF, PCOLS = _pack_offsets(NCH)
    W128 = W16 // 8
    pack_d = nc.dram_tensor("pack", [128, PCOLS], I8, kind="ExternalInput")
    out_d = nc.dram_tensor("out", [G, CLASSES], FP, kind="ExternalOutput")

    with tile.TileContext(nc) as tc:
        with (
            tc.tile_pool(name="const", bufs=1) as cp,
            tc.tile_pool(name="work", bufs=2) as wp,
            tc.tile_pool(name="psmm", bufs=3, space="PSUM") as pmm,
            tc.tile_pool(name="pstr", bufs=2, space="PSUM") as ptr,
            tc.tile_pool(name="psro", bufs=2, space="PSUM") as pro,
            tc.tile_pool(name="dram", bufs=1, space="DRAM") as dp,
        ):
            # ---- persistent tiles ----
            idx_t = cp.tile([128, W16], I16)
            slot8_t = cp.tile([128, NCH], I8)
            slot_t = cp.tile([128, NCH], BF)
            misc_t = cp.tile([128, MCOLS], FP)
            gslot_t = cp.tile([128, GRP], BF)
            xsb_t = cp.tile([128, GRP], BF)
            iota_b = cp.tile([128, 128], BF)
            iota_f = cp.tile([128, 128], FP)
            ident_b = cp.tile([128, 128], BF)
            ident_f = cp.tile([128, 128], FP)
            ones_b = cp.tile([128, 1], BF)
            normc_t = cp.tile([128, GRP], FP)
            normb_t = cp.tile([128, GRP], BF)
            w_t = [cp.tile([128, HOPS + 1, F], BF, name=f"w{l}_t", tag=f"w{l}")
                   for l in range(HLAYERS + 1)]
            f0T = cp.tile([128, GRP * 128], BF)   # feat-major [f, i] per group
            f1T = cp.tile([128, GRP * 128], BF)
            f2T = cp.tile([128, GRP * 128], BF)
            roacc_t = cp.tile([128, F + 1], FP)
            ro2_t = cp.tile([128, F + 1], FP)
            cnt_t = cp.tile([128, 1], FP)
            rcp_t = cp.tile([128, 1], FP)
            hg_t = cp.tile([128, F], FP)
            hgT_t = cp.tile([F, 128], FP)
            logit_t = cp.tile([128, CLASSES], FP)

            T_in = dp.tile([NT, F], BF)
            T_hop = dp.tile([NT, F], BF)
            ag_in = dp.tile([NPAD, F], BF)
            ar_in = dp.tile([128, F + 1], FP)
            ar_out = dp.tile([128, F + 1], FP)

            # ---- constants ----
            # idx arrives as [128, W128] i16 bytes where row 16a+b holds
            # idx_c[b, a*W128 : (a+1)*W128]; expand to the gather's
            # [128, W16] layout (16-partition wrap replicated 8x).
            for a in range(8):
                for p in range(8):
                    nc.sync.dma_start(
                        idx_t[p * 16 : (p + 1) * 16, a * W128 : (a + 1) * W128],
                        pack_d[16 * a : 16 * a + 16,
                               IDX_OFF : IDX_OFF + W128 * 2].bitcast(I16))
            nc.sync.dma_start(slot8_t[:], pack_d[:, SLOT_OFF : SLOT_OFF + NCH])
            nc.sync.dma_start(
                misc_t[:], pack_d[:, MISC_OFF : MISC_OFF + MCOLS * 4].bitcast(FP))
            nc.vector.tensor_copy(slot_t[:], slot8_t[:])
            nc.vector.tensor_copy(gslot_t[:], misc_t[:, MC_GS : MC_GS + GRP])
            nc.vector.tensor_copy(xsb_t[:], misc_t[:, MC_XS : MC_XS + GRP])
            for l in range(HLAYERS + 1):
                for k in range(HOPS + 1):
                    c0 = W_OFF + (l * (HOPS + 1) + k) * F * 2
                    nc.sync.dma_start(w_t[l][:, k, :],
                                      pack_d[:, c0 : c0 + F * 2].bitcast(BF))

            nc.gpsimd.iota(iota_f[:], pattern=[[1, 128]], base=0, channel_multiplier=0,
                           allow_small_or_imprecise_dtypes=True)
            nc.vector.tensor_copy(iota_b[:], iota_f[:])
            icol_t = cp.tile([128, 1], FP)
            nc.gpsimd.iota(icol_t[:], pattern=[[0, 1]], base=0, channel_multiplier=1,
                           allow_small_or_imprecise_dtypes=True)
            nc.vector.tensor_tensor(ident_f[:], icol_t[:].broadcast_to([128, 128]),
                                    iota_f[:], mybir.AluOpType.is_equal)
            nc.vector.tensor_copy(ident_b[:], ident_f[:])
            nc.vector.memset(ones_b[:], 1.0)
            nc.vector.memset(roacc_t[:], 0.0)

            def bail():
                nc.vector.tensor_copy(logit_t[:], iota_f[:, :CLASSES])
                nc.sync.dma_start(out_d[:, :], logit_t[:])

            def onehot(g):
                """[128e, CH, 128j] one-hot tile for group g (one DVE op)."""
                oh = wp.tile([128, CH, 128], BF, name="oh", tag="oh")
                nc.vector.tensor_tensor(
                    oh[:, :, :],
                    slot_t[:, ds(g * CH, CH)].unsqueeze(2).broadcast_to([128, CH, 128]),
                    iota_b[:].unsqueeze(1).broadcast_to([128, CH, 128]),
                    mybir.AluOpType.is_equal,
                )
                return oh

            # ---- degree / norm pass ----
            with tc.For_i(0, GRP, 1) as g:
                oh = onehot(g)
                dps = pmm.tile([128, 128], FP, name="dps", tag="mm")
                for c in range(CH):
                    nc.tensor.matmul(dps[:, 0:1], oh[:, c, :], ones_b[:],
                                     start=(c == 0), stop=(c == CH - 1))
                dmx = wp.tile([128, 1], FP, name="dmx", tag="dmx")
                nc.vector.tensor_scalar_max(dmx[:], dps[:, 0:1], 1.0)
                drc = wp.tile([128, 1], FP, name="drc", tag="drc")
                nc.vector.reciprocal(drc[:], dmx[:])
                nc.scalar.activation(normc_t[:, ds(g, 1)], drc[:],
                                     mybir.ActivationFunctionType.Sqrt)
            nc.vector.tensor_copy(normb_t[:], normc_t[:])
            STOP = LVL <= ORDER.index("deg")
            if STOP:
                bail()

            # ---- T0 = x * norm ; f0T = x^T ----
            if not STOP:
                with tc.For_i(0, GRP, 1) as g:
                    x8 = wp.tile([128, F], I8, name="x8", tag="x8")
                    nc.sync.dma_start(x8[:], pack_d[:, ds(g * F, F)])
                    xb = wp.tile([128, F], BF, name="xb", tag="xb")
                    nc.vector.tensor_copy(xb[:], x8[:])
                    xt = wp.tile([128, F], BF, name="xt", tag="xt")
                    nc.vector.tensor_tensor(
                        xt[:], xb[:], xsb_t[:, ds(g, 1)].broadcast_to([128, F]),
                        mybir.AluOpType.mult)
                    t0 = wp.tile([128, F], BF, name="t0", tag="tn")
                    nc.vector.tensor_tensor(
                        t0[:], xt[:], normb_t[:, ds(g, 1)].broadcast_to([128, F]),
                        mybir.AluOpType.mult)
                    nc.sync.dma_start(ag_in[ds(g * 128, 128), :], t0[:])
                    pt = ptr.tile([128, 128], BF, name="pt", tag="tr")
                    nc.tensor.transpose(pt[:], xt[:], ident_b[:])
                    nc.vector.tensor_copy(f0T[:, ds(g * 128, 128)], pt[:])
            if not STOP and LVL <= ORDER.index("t0"):
                bail()
                STOP = True
            if not STOP:
                nc.gpsimd.collective_compute(
                    "AllGather", mybir.AluOpType.bypass, replica_groups=RG,
                    ins=[ag_in.opt()], outs=[T_in.opt()])
            if not STOP and LVL <= ORDER.index("ag0"):
                bail()
                STOP = True

            def hop(src_tbl, fT, make_table):
                """One SpMM hop: gather -> one-hot segsum -> scale; optionally
                also emit next scaled table slice into ag_in."""
                with tc.For_i(0, GRP, 1) as g:
                    vb = wp.tile([128, CH, 128], BF, name="vb", tag="vb")
                    nc.gpsimd.dma_gather(
                        vb[:, 0:CAu, :], src_tbl[:, :],
                        idx_t[:, ds(g * CH * 8, CAu * 8)],
                        CAu * 128, CAu * 128, F, single_packet=False)
                    nc.gpsimd.dma_gather(
                        vb[:, CAu:CH, :], src_tbl[HALF:, :],
                        idx_t[:, ds(g * CH * 8 + CAu * 8, CBu * 8)],
                        CBu * 128, CBu * 128, F, single_packet=False)
                    oh = onehot(g)
                    ps = pmm.tile([128, 128], FP, name="ps", tag="mm")
                    for c in range(CH):
                        nc.tensor.matmul(ps[:], oh[:, c, :], vb[:, c, :],
                                         start=(c == 0), stop=(c == CH - 1))
                    fn = wp.tile([128, F], BF, name="fn", tag="fn")
                    nc.vector.tensor_tensor(
                        fn[:], ps[:], normc_t[:, ds(g, 1)].broadcast_to([128, F]),
                        mybir.AluOpType.mult)
                    if make_table:
                        tn = wp.tile([128, F], BF, name="tn", tag="tn")
                        nc.vector.tensor_tensor(
                            tn[:], fn[:], normb_t[:, ds(g, 1)].broadcast_to([128, F]),
                            mybir.AluOpType.mult)
                        nc.sync.dma_start(ag_in[ds(g * 128, 128), :], tn[:])
                    pt = ptr.tile([128, 128], BF, name="pt2", tag="tr")
                    nc.tensor.transpose(pt[:], fn[:], ident_b[:])
                    nc.vector.tensor_copy(fT[:, ds(g * 128, 128)], pt[:])

            for l in range(HLAYERS + 1) if not STOP else []:
                hop(T_in, f1T, make_table=True)
                if l == 0 and LVL <= ORDER.index("hop1"):
                    bail()
                    STOP = True
                    break
                nc.gpsimd.collective_compute(
                    "AllGather", mybir.AluOpType.bypass, replica_groups=RG,
                    ins=[ag_in.opt()], outs=[T_hop.opt()])
                if l == 0 and LVL <= ORDER.index("aghop"):
                    bail()
                    STOP = True
                    break
                hop(T_hop, f2T, make_table=False)
                if l == 0 and LVL <= ORDER.index("hop2"):
                    bail()
                    STOP = True
                    break
                fTs = [f0T, f1T, f2T]
                with tc.For_i(0, GRP, 1) as g:
                    ph = pmm.tile([128, 128], FP, name="ph", tag="mm")
                    for k in range(HOPS + 1):
                        nc.tensor.matmul(ph[:], w_t[l][:, k, :],
                                         fTs[k][:, ds(g * 128, 128)],
                                         start=(k == 0), stop=(k == HOPS))
                    act = wp.tile([128, 128], BF, name="act", tag="act")
                    nc.scalar.activation(act[:], ph[:],
                                         mybir.ActivationFunctionType.Relu,
                                         bias=misc_t[:, MC_B + l : MC_B + l + 1])
                    nc.vector.tensor_copy(f0T[:, ds(g * 128, 128)], act[:])
                    pt = ptr.tile([128, 128], BF, name="pt3", tag="tr")
                    nc.tensor.transpose(pt[:], act[:], ident_b[:])
                    if l < HLAYERS:
                        tn = wp.tile([128, F], BF, name="tn2", tag="tn")
                        nc.vector.tensor_tensor(
                            tn[:], pt[:], normb_t[:, ds(g, 1)].broadcast_to([128, F]),
                            mybir.AluOpType.mult)
                        nc.sync.dma_start(ag_in[ds(g * 128, 128), :], tn[:])
                    else:
                        rr = wp.tile([128, F + 1], BF, name="rr", tag="rr")
                        nc.vector.tensor_copy(rr[:, 0:F], pt[:])
                        nc.vector.tensor_copy(rr[:, F : F + 1], ones_b[:])
                        og = wp.tile([128, 128], BF, name="og", tag="og")
                        nc.vector.tensor_tensor(
                            og[:], gslot_t[:, ds(g, 1)].broadcast_to([128, 128]),
                            iota_b[:], mybir.AluOpType.is_equal)
                        pr = pro.tile([128, F + 1], FP, name="pr", tag="ro")
                        nc.tensor.matmul(pr[:], og[:], rr[:], start=True, stop=True)
                        nc.vector.tensor_tensor(roacc_t[:], roacc_t[:], pr[:],
                                                mybir.AluOpType.add)
                if l < HLAYERS:
                    nc.gpsimd.collective_compute(
                        "AllGather", mybir.AluOpType.bypass, replica_groups=RG,
                        ins=[ag_in.opt()], outs=[T_in.opt()])
                if l == 0 and LVL <= ORDER.index("layer0"):
                    bail()
                    STOP = True
                    break

            # ---- readout: all-reduce partial sums, mean, classify ----
            if not STOP:
                nc.sync.dma_start(ar_in[:, :], roacc_t[:])
                nc.gpsimd.collective_compute(
                    "AllReduce", mybir.AluOpType.add, replica_groups=RG,
                    ins=[ar_in.opt()], outs=[ar_out.opt()])
                nc.sync.dma_start(ro2_t[:], ar_out[:, :])
                nc.vector.tensor_scalar_max(cnt_t[:], ro2_t[:, F : F + 1], 1.0)
                nc.vector.reciprocal(rcp_t[:], cnt_t[:])
                nc.vector.tensor_tensor(hg_t[:], ro2_t[:, 0:F],
                                        rcp_t[:].broadcast_to([128, F]),
                                        mybir.AluOpType.mult)
                ptf = ptr.tile([128, 128], FP, name="ptf", tag="tr")
                nc.tensor.transpose(ptf[:], hg_t[:], ident_f[:])
                nc.vector.tensor_copy(hgT_t[:], ptf[:])
                plog = pro.tile([128, F + 1], FP, name="plog", tag="ro")
                nc.tensor.matmul(plog[:, 0:CLASSES], hgT_t[:],
                                 misc_t[:, MC_WC : MC_WC + CLASSES],
                                 start=True, stop=True)
                nc.vector.tensor_tensor(logit_t[:], plog[:, 0:CLASSES],
                                        misc_t[:, MC_BC : MC_BC + CLASSES],
                                        mybir.AluOpType.add)
                nc.sync.dma_start(out_d[:, :], logit_t[:])

    nc.finalize()
    return nc


def _make_in_maps(x, graph_ids, Ws, bs, Wc, bc, idx_c, slot_cols):
    b_cols = np.stack(bs, axis=1).astype(np.float32)            # [128, 3]
    bc_rep = np.tile(np.asarray(bc, np.float32)[None, :], (128, 1))
    w_bf = np.concatenate([np.asarray(w, np.float32) for w in Ws], axis=0).astype(NPBF)
    wc_f = np.asarray(Wc, np.float32)
    # per-node int8 quantization of x
    x_full8 = np.zeros((N, F), np.int8)
    xs_full = np.ones(N, np.float32)
    amax = np.abs(x).max(axis=1)
    nz = amax > 0
    xs_full[nz] = amax[nz] / 127.0
    x_full8 = np.clip(np.round(x / xs_full[:, None]), -127, 127).astype(np.int8)
    # weights packed slot-major: [128, 9*256] bytes
    w_pack = np.ascontiguousarray(
        w_bf.reshape(3 * (HOPS + 1), 128, F).transpose(1, 0, 2)
    ).view(np.int8).reshape(128, -1)
    in_maps = []
    for c in range(NCORES):
        x_loc = np.zeros((NPAD, F), np.int8)
        x_loc[:PER] = x_full8[c * PER : (c + 1) * PER]
        x_pack = np.ascontiguousarray(
            x_loc.reshape(GRP, 128, F).transpose(1, 0, 2)).reshape(128, GRP * F)
        xs = np.ones(NPAD, np.float32)
        xs[:PER] = xs_full[c * PER : (c + 1) * PER]
        gsl = np.full(NPAD, 255.0, np.float32)
        gsl[:PER] = graph_ids[c * PER : (c + 1) * PER].astype(np.float32)
        misc = np.concatenate([
            b_cols, wc_f, bc_rep,
            np.ascontiguousarray(xs.reshape(GRP, 128).T),
            np.ascontiguousarray(gsl.reshape(GRP, 128).T),
        ], axis=1).astype(np.float32)
        W16 = idx_c.shape[2]
        idx_pack = np.ascontiguousarray(
            idx_c[c].reshape(16, 8, W16 // 8).transpose(1, 0, 2)
        ).reshape(128, W16 // 8).view(np.int8)
        parts = [x_pack, slot_cols[c].view(np.int8), w_pack,
                 np.ascontiguousarray(misc).view(np.int8), idx_pack]
        X_OFF, SLOT_OFF, W_OFF, MISC_OFF, IDX_OFF, PCOLS = _pack_offsets(
            slot_cols.shape[2])
        pack = np.zeros((128, PCOLS), np.int8)
        for p, o in zip(parts, (X_OFF, SLOT_OFF, W_OFF, MISC_OFF, IDX_OFF)):
            pack[:, o : o + p.shape[1]] = p
        in_maps.append(dict(pack=pack))
    return in_maps


def _pack_offsets(NCH):
    MCOLS = 3 + CLASSES + CLASSES + GRP + GRP
    W16 = NCH * 128 // 16
    SLOT_OFF = GRP * F
    W_OFF = -(-(SLOT_OFF + NCH) // 4) * 4
    MISC_OFF = -(-(W_OFF + (HLAYERS + 1) * (HOPS + 1) * F * 2) // 4) * 4
    IDX_OFF = MISC_OFF + MCOLS * 4
    PCOLS = IDX_OFF + (W16 // 8) * 2
    return 0, SLOT_OFF, W_OFF, MISC_OFF, IDX_OFF, PCOLS


def kernel(x, src, dst, graph_ids, W0, b0, W1, b1, W2, b2, Wc, bc, **_):
    x = np.asarray(x, np.float32)
    graph_ids = np.asarray(graph_ids, np.int64)
    idx_c, slot_cols, CAu, CBu = _prep_edges(src, dst)
    nc = _build_program(CAu, CBu)
    in_maps = _make_in_maps(
        x, graph_ids,
        [np.asarray(W0), np.asarray(W1), np.asarray(W2)],
        [np.asarray(b0, np.float32), np.asarray(b1, np.float32),
         np.asarray(b2, np.float32)],
        Wc, bc, idx_c, slot_cols)
    res = bass_utils.run_bass_kernel_spmd(nc, in_maps, core_ids=list(range(NCORES)))
    return np.asarray(res.results[0]["out"], np.float32)


# revision 38
# speedup vs baseline: 18.5756x; 1.2295x over previous
"""TAGConv GNN classifier on 8 Trainium2 NeuronCores.

Sharding: nodes split into 8 contiguous slices (6250/core, padded to 6272);
edges live on the core that owns their dst. Each hop: every core gathers
src rows from a replicated norm-prescaled bf16 node table in HBM
(dma_gather, int16 indices -> split-table trick), segment-sums them into
its dst slice with one-hot matmuls on TensorE (PSUM accumulation), rescales
by norm, and all-gathers its slice of the next table. Readout partial sums
per graph are all-reduced, then every core computes the (identical) logits.

Per-group work is uniform (chunk counts padded to the max) so each pass is
a single hardware For_i loop over the 49 dst groups -> ~40x fewer
instructions than full unrolling, which cuts per-call program
serialize/load overhead. Inputs are uploaded compactly (bf16 features and
weights, de-replicated int16 gather indices, uint8 slots/graph-ids) to cut
host->device transfer bytes.
"""
import os
import tempfile

import numpy as np
import ml_dtypes

import jax

# Persistent XLA compilation cache: run_bass_kernel_spmd builds a fresh jit
# per call, so without this every call re-runs the PJRT compile (~130ms via
# the axon tunnel). With it, repeat compiles deserialize from disk (~8ms).
try:
    jax.config.update(
        "jax_compilation_cache_dir",
        os.path.join(tempfile.gettempdir(), "jax_comp_cache"))
    jax.config.update("jax_persistent_cache_min_entry_size_bytes", 0)
    jax.config.update("jax_persistent_cache_min_compile_time_secs", 0.0)
except Exception:
    pass

import concourse.bass as bass
import concourse.bacc as bacc
import concourse.mybir as mybir
import concourse.tile as tile
from concourse import bass_utils
from concourse.bass import ds

N, E, G = 50000, 800000, 128
F = 128                      # IN_DIM == HID
CLASSES = 10
HOPS, HLAYERS = 2, 2         # 3 TAGConv layers total
NCORES = 8

PER = N // NCORES            # real nodes per core
GRP = (PER + 127) // 128     # dst groups of 128 per core
NPAD = GRP * 128             # padded nodes per core
NT = NCORES * NPAD           # padded total
HALF = NT // 2               # int16-safe split of the node table

FP = mybir.dt.float32
BF = mybir.dt.bfloat16
I16 = mybir.dt.int16
U8 = mybir.dt.uint8
NPBF = ml_dtypes.bfloat16


def _prep_edges(src, dst):
    """Per-core gather-index + slot tables with uniform chunks per group."""
    src = np.asarray(src).astype(np.int64)
    dst = np.asarray(dst).astype(np.int64)
    core = dst // PER
    local = dst - core * PER
    grp = local // 128
    slot = local % 128
    ps = (src // PER) * NPAD + (src % PER)          # padded global src id
    half = (ps >= HALF).astype(np.int64)
    idxv = ps - half * HALF                          # int16-safe index

    key = (core * GRP + grp) * 2 + half
    order = np.argsort(key, kind="stable")
    cnt = np.bincount(key, minlength=NCORES * GRP * 2).reshape(NCORES, GRP, 2)
    CAu = max(1, -(-int(cnt[:, :, 0].max()) // 128))
    CBu = max(1, -(-int(cnt[:, :, 1].max()) // 128))
    CH = CAu + CBu
    TOT = GRP * CH * 128

    idx16 = np.zeros((NCORES, TOT), np.int16)
    slotu = np.full((NCORES, TOT), 255, np.uint8)
    sidx = idxv[order]
    sslot = slot[order].astype(np.uint8)
    starts = np.concatenate([[0], np.cumsum(cnt.reshape(-1))]).astype(int)
    for c in range(NCORES):
        for g in range(GRP):
            base = g * CH * 128
            for h, off in ((0, base), (1, base + CAu * 128)):
                k = (c * GRP + g) * 2 + h
                n = int(cnt[c, g, h])
                s0 = starts[k]
                idx16[c, off : off + n] = sidx[s0 : s0 + n]
                slotu[c, off : off + n] = sslot[s0 : s0 + n]

    idx_c = np.ascontiguousarray(idx16.reshape(NCORES, -1, 16).transpose(0, 2, 1))
    slot_cols = np.ascontiguousarray(
        slotu.reshape(NCORES, GRP * CH, 128).transpose(0, 2, 1))
    return idx_c, slot_cols, CAu, CBu


def _build_program(CAu, CBu):
    STAGE = os.environ.get("KSTAGE", "full")
    ORDER = ["deg", "t0", "ag0", "hop1", "aghop", "hop2", "layer0", "full"]
    LVL = ORDER.index(STAGE)
    CH = CAu + CBu
    NCH = GRP * CH
    TOT = NCH * 128
    W16 = TOT // 16
    nc = bacc.Bacc("TRN2", target_bir_lowering=False, debug=False, num_devices=NCORES)
    RG = [list(range(NCORES))]

    I8 = mybir.dt.int8
    # misc layout (fp32 columns): [0:3]=b_cols, [3:13]=wc, [13:23]=bc_rep,
    # [23:+GRP]=x row scales (slot-major), then -32*scale, then gslot as fp32.
    MC_B, MC_WC, MC_BC = 0, 3, 3 + CLASSES
    MC_XS = MC_BC + CLASSES
    MC_XB = MC_XS + GRP
    MC_GS = MC_XB + GRP
    MCOLS = MC_GS + GRP
    # single packed i8 input: x (slot-major blocks), slots, weights, misc, idx
    X_OFF, SLOT_OFF, W_OFF, MISC_OFF, IDX_OFF, PCOLS = _pack_offsets(NCH)
    W128 = W16 // 8
    pack_d = nc.dram_tensor("pack", [128, PCOLS], I8, kind="ExternalInput")
    out_d = nc.dram_tensor("out", [G, CLASSES], FP, kind="ExternalOutput")

    with tile.TileContext(nc) as tc:
        with (
            tc.tile_pool(name="const", bufs=1) as cp,
            tc.tile_pool(name="work", bufs=2) as wp,
            tc.tile_pool(name="psmm", bufs=3, space="PSUM") as pmm,
            tc.tile_pool(name="pstr", bufs=2, space="PSUM") as ptr,
            tc.tile_pool(name="psro", bufs=2, space="PSUM") as pro,
            tc.tile_pool(name="dram", bufs=1, space="DRAM") as dp,
        ):
            # ---- persistent tiles ----
            idx_t = cp.tile([128, W16], I16)
            slot8_t = cp.tile([128, NCH], I8)
            slot_t = cp.tile([128, NCH], BF)
            misc_t = cp.tile([128, MCOLS], FP)
            gslot_t = cp.tile([128, GRP], BF)
            xsb_t = cp.tile([128, GRP], BF)
            xbb_t = cp.tile([128, GRP], BF)
            iota_b = cp.tile([128, 128], BF)
            iota_f = cp.tile([128, 128], FP)
            ident_b = cp.tile([128, 128], BF)
            ident_f = cp.tile([128, 128], FP)
            ones_b = cp.tile([128, 1], BF)
            normc_t = cp.tile([128, GRP], FP)
            normb_t = cp.tile([128, GRP], BF)
            w_t = [cp.tile([128, HOPS + 1, F], BF, name=f"w{l}_t", tag=f"w{l}")
                   for l in range(HLAYERS + 1)]
            f0T = cp.tile([128, GRP * 128], BF)   # feat-major [f, i] per group
            f1T = cp.tile([128, GRP * 128], BF)
            f2T = cp.tile([128, GRP * 128], BF)
            roacc_t = cp.tile([128, F + 1], FP)
            ro2_t = cp.tile([128, F + 1], FP)
            cnt_t = cp.tile([128, 1], FP)
            rcp_t = cp.tile([128, 1], FP)
            hg_t = cp.tile([128, F], FP)
            hgT_t = cp.tile([F, 128], FP)
            logit_t = cp.tile([128, CLASSES], FP)

            T_in = dp.tile([NT, F], BF)
            T_hop = dp.tile([NT, F], BF)
            ag_in = dp.tile([NPAD, F], BF)
            ar_in = dp.tile([128, F + 1], FP)
            ar_out = dp.tile([128, F + 1], FP)

            # ---- constants ----
            # idx arrives as [128, W128] i16 bytes where row 16a+b holds
            # idx_c[b, a*W128 : (a+1)*W128]; expand to the gather's
            # [128, W16] layout (16-partition wrap replicated 8x).
            for a in range(8):
                for p in range(8):
                    nc.sync.dma_start(
                        idx_t[p * 16 : (p + 1) * 16, a * W128 : (a + 1) * W128],
                        pack_d[16 * a : 16 * a + 16,
                               IDX_OFF : IDX_OFF + W128 * 2].bitcast(I16))
            nc.sync.dma_start(slot8_t[:], pack_d[:, SLOT_OFF : SLOT_OFF + NCH])
            nc.sync.dma_start(
                misc_t[:], pack_d[:, MISC_OFF : MISC_OFF + MCOLS * 4].bitcast(FP))
            nc.vector.tensor_copy(slot_t[:], slot8_t[:])
            nc.vector.tensor_copy(gslot_t[:], misc_t[:, MC_GS : MC_GS + GRP])
            nc.vector.tensor_copy(xsb_t[:], misc_t[:, MC_XS : MC_XS + GRP])
            nc.vector.tensor_copy(xbb_t[:], misc_t[:, MC_XB : MC_XB + GRP])
            for l in range(HLAYERS + 1):
                for k in range(HOPS + 1):
                    c0 = W_OFF + (l * (HOPS + 1) + k) * F * 2
                    nc.sync.dma_start(w_t[l][:, k, :],
                                      pack_d[:, c0 : c0 + F * 2].bitcast(BF))

            nc.gpsimd.iota(iota_f[:], pattern=[[1, 128]], base=0, channel_multiplier=0,
                           allow_small_or_imprecise_dtypes=True)
            nc.vector.tensor_copy(iota_b[:], iota_f[:])
            icol_t = cp.tile([128, 1], FP)
            nc.gpsimd.iota(icol_t[:], pattern=[[0, 1]], base=0, channel_multiplier=1,
                           allow_small_or_imprecise_dtypes=True)
            nc.vector.tensor_tensor(ident_f[:], icol_t[:].broadcast_to([128, 128]),
                                    iota_f[:], mybir.AluOpType.is_equal)
            nc.vector.tensor_copy(ident_b[:], ident_f[:])
            nc.vector.memset(ones_b[:], 1.0)
            nc.vector.memset(roacc_t[:], 0.0)

            def bail():
                nc.vector.tensor_copy(logit_t[:], iota_f[:, :CLASSES])
                nc.sync.dma_start(out_d[:, :], logit_t[:])

            def onehot(g):
                """[128e, CH, 128j] one-hot tile for group g (one DVE op)."""
                oh = wp.tile([128, CH, 128], BF, name="oh", tag="oh")
                nc.vector.tensor_tensor(
                    oh[:, :, :],
                    slot_t[:, ds(g * CH, CH)].unsqueeze(2).broadcast_to([128, CH, 128]),
                    iota_b[:].unsqueeze(1).broadcast_to([128, CH, 128]),
                    mybir.AluOpType.is_equal,
                )
                return oh

            # ---- degree / norm pass ----
            with tc.For_i(0, GRP, 1) as g:
                oh = onehot(g)
                dps = pmm.tile([128, 128], FP, name="dps", tag="mm")
                for c in range(CH):
                    nc.tensor.matmul(dps[:, 0:1], oh[:, c, :], ones_b[:],
                                     start=(c == 0), stop=(c == CH - 1))
                dmx = wp.tile([128, 1], FP, name="dmx", tag="dmx")
                nc.vector.tensor_scalar_max(dmx[:], dps[:, 0:1], 1.0)
                drc = wp.tile([128, 1], FP, name="drc", tag="drc")
                nc.vector.reciprocal(drc[:], dmx[:])
                nc.scalar.activation(normc_t[:, ds(g, 1)], drc[:],
                                     mybir.ActivationFunctionType.Sqrt)
            nc.vector.tensor_copy(normb_t[:], normc_t[:])
            STOP = LVL <= ORDER.index("deg")
            if STOP:
                bail()

            # ---- T0 = x * norm ; f0T = x^T ----
            # x arrives int6 offset-binary, 4 values packed little-endian in
            # 3 bytes; unpacked column order is j*32+k for value j of quad k
            # (host permutes W0's input rows to match).
            if not STOP:
                Q = F // 4   # quads
                with tc.For_i(0, GRP, 1) as g:
                    x8 = wp.tile([128, XB], U8, name="x8", tag="x8")
                    nc.sync.dma_start(x8[:], pack_d[:, ds(g * XB, XB)].bitcast(U8))
                    B0, B1, B2 = (x8[:, i * Q : (i + 1) * Q] for i in range(3))
                    qt = wp.tile([128, F], U8, name="qt", tag="qt")
                    tq = wp.tile([128, 4, Q], U8, name="tq", tag="tq")
                    nc.vector.tensor_scalar(qt[:, 0:Q], B0, 63, None,
                                            mybir.AluOpType.bitwise_and)
                    nc.vector.tensor_scalar(tq[:, 0, :], B0, 6, None,
                                            mybir.AluOpType.logical_shift_right)
                    nc.vector.tensor_scalar(tq[:, 1, :], B1, 15, None,
                                            mybir.AluOpType.bitwise_and)
                    nc.vector.tensor_scalar(tq[:, 1, :], tq[:, 1, :], 2, None,
                                            mybir.AluOpType.logical_shift_left)
                    nc.vector.tensor_tensor(qt[:, Q : 2 * Q], tq[:, 0, :],
                                            tq[:, 1, :], mybir.AluOpType.bitwise_or)
                    nc.vector.tensor_scalar(tq[:, 2, :], B1, 4, None,
                                            mybir.AluOpType.logical_shift_right)
                    nc.vector.tensor_scalar(tq[:, 3, :], B2, 3, None,
                                            mybir.AluOpType.bitwise_and)
                    nc.vector.tensor_scalar(tq[:, 3, :], tq[:, 3, :], 4, None,
                                            mybir.AluOpType.logical_shift_left)
                    nc.vector.tensor_tensor(qt[:, 2 * Q : 3 * Q], tq[:, 2, :],
                                            tq[:, 3, :], mybir.AluOpType.bitwise_or)
                    nc.vector.tensor_scalar(qt[:, 3 * Q : 4 * Q], B2, 2, None,
                                            mybir.AluOpType.logical_shift_right)
                    xb = wp.tile([128, F], BF, name="xb", tag="xb")
                    nc.vector.tensor_copy(xb[:], qt[:])
                    xs = wp.tile([128, F], BF, name="xs", tag="xs")
                    nc.vector.tensor_tensor(
                        xs[:], xb[:], xsb_t[:, ds(g, 1)].broadcast_to([128, F]),
                        mybir.AluOpType.mult)
                    xt = wp.tile([128, F], BF, name="xt", tag="xt")
                    nc.vector.tensor_tensor(
                        xt[:], xs[:], xbb_t[:, ds(g, 1)].broadcast_to([128, F]),
                        mybir.AluOpType.add)
                    t0 = wp.tile([128, F], BF, name="t0", tag="tn")
                    nc.vector.tensor_tensor(
                        t0[:], xt[:], normb_t[:, ds(g, 1)].broadcast_to([128, F]),
                        mybir.AluOpType.mult)
                    nc.sync.dma_start(ag_in[ds(g * 128, 128), :], t0[:])
                    pt = ptr.tile([128, 128], BF, name="pt", tag="tr")
                    nc.tensor.transpose(pt[:], xt[:], ident_b[:])
                    nc.vector.tensor_copy(f0T[:, ds(g * 128, 128)], pt[:])
            if not STOP and LVL <= ORDER.index("t0"):
                bail()
                STOP = True
            if not STOP:
                nc.gpsimd.collective_compute(
                    "AllGather", mybir.AluOpType.bypass, replica_groups=RG,
                    ins=[ag_in.opt()], outs=[T_in.opt()])
            if not STOP and LVL <= ORDER.index("ag0"):
                bail()
                STOP = True

            def hop(src_tbl, fT, make_table):
                """One SpMM hop: gather -> one-hot segsum -> scale; optionally
                also emit next scaled table slice into ag_in."""
                with tc.For_i(0, GRP, 1) as g:
                    vb = wp.tile([128, CH, 128], BF, name="vb", tag="vb")
                    nc.gpsimd.dma_gather(
                        vb[:, 0:CAu, :], src_tbl[:, :],
                        idx_t[:, ds(g * CH * 8, CAu * 8)],
                        CAu * 128, CAu * 128, F, single_packet=False)
                    nc.gpsimd.dma_gather(
                        vb[:, CAu:CH, :], src_tbl[HALF:, :],
                        idx_t[:, ds(g * CH * 8 + CAu * 8, CBu * 8)],
                        CBu * 128, CBu * 128, F, single_packet=False)
                    oh = onehot(g)
                    ps = pmm.tile([128, 128], FP, name="ps", tag="mm")
                    for c in range(CH):
                        nc.tensor.matmul(ps[:], oh[:, c, :], vb[:, c, :],
                                         start=(c == 0), stop=(c == CH - 1))
                    fn = wp.tile([128, F], BF, name="fn", tag="fn")
                    nc.vector.tensor_tensor(
                        fn[:], ps[:], normc_t[:, ds(g, 1)].broadcast_to([128, F]),
                        mybir.AluOpType.mult)
                    if make_table:
                        tn = wp.tile([128, F], BF, name="tn", tag="tn")
                        nc.vector.tensor_tensor(
                            tn[:], fn[:], normb_t[:, ds(g, 1)].broadcast_to([128, F]),
                            mybir.AluOpType.mult)
                        nc.sync.dma_start(ag_in[ds(g * 128, 128), :], tn[:])
                    pt = ptr.tile([128, 128], BF, name="pt2", tag="tr")
                    nc.tensor.transpose(pt[:], fn[:], ident_b[:])
                    nc.vector.tensor_copy(fT[:, ds(g * 128, 128)], pt[:])

            for l in range(HLAYERS + 1) if not STOP else []:
                hop(T_in, f1T, make_table=True)
                if l == 0 and LVL <= ORDER.index("hop1"):
                    bail()
                    STOP = True
                    break
                nc.gpsimd.collective_compute(
                    "AllGather", mybir.AluOpType.bypass, replica_groups=RG,
                    ins=[ag_in.opt()], outs=[T_hop.opt()])
                if l == 0 and LVL <= ORDER.index("aghop"):
                    bail()
                    STOP = True
                    break
                hop(T_hop, f2T, make_table=False)
                if l == 0 and LVL <= ORDER.index("hop2"):
                    bail()
                    STOP = True
                    break
                fTs = [f0T, f1T, f2T]
                with tc.For_i(0, GRP, 1) as g:
                    ph = pmm.tile([128, 128], FP, name="ph", tag="mm")
                    for k in range(HOPS + 1):
                        nc.tensor.matmul(ph[:], w_t[l][:, k, :],
                                         fTs[k][:, ds(g * 128, 128)],
                                         start=(k == 0), stop=(k == HOPS))
                    act = wp.tile([128, 128], BF, name="act", tag="act")
                    nc.scalar.activation(act[:], ph[:],
                                         mybir.ActivationFunctionType.Relu,
                                         bias=misc_t[:, MC_B + l : MC_B + l + 1])
                    nc.vector.tensor_copy(f0T[:, ds(g * 128, 128)], act[:])
                    pt = ptr.tile([128, 128], BF, name="pt3", tag="tr")
                    nc.tensor.transpose(pt[:], act[:], ident_b[:])
                    if l < HLAYERS:
                        tn = wp.tile([128, F], BF, name="tn2", tag="tn")
                        nc.vector.tensor_tensor(
                            tn[:], pt[:], normb_t[:, ds(g, 1)].broadcast_to([128, F]),
                            mybir.AluOpType.mult)
                        nc.sync.dma_start(ag_in[ds(g * 128, 128), :], tn[:])
                    else:
                        rr = wp.tile([128, F + 1], BF, name="rr", tag="rr")
                        nc.vector.tensor_copy(rr[:, 0:F], pt[:])
                        nc.vector.tensor_copy(rr[:, F : F + 1], ones_b[:])
                        og = wp.tile([128, 128], BF, name="og", tag="og")
                        nc.vector.tensor_tensor(
                            og[:], gslot_t[:, ds(g, 1)].broadcast_to([128, 128]),
                            iota_b[:], mybir.AluOpType.is_equal)
                        pr = pro.tile([128, F + 1], FP, name="pr", tag="ro")
                        nc.tensor.matmul(pr[:], og[:], rr[:], start=True, stop=True)
                        nc.vector.tensor_tensor(roacc_t[:], roacc_t[:], pr[:],
                                                mybir.AluOpType.add)
                if l < HLAYERS:
                    nc.gpsimd.collective_compute(
                        "AllGather", mybir.AluOpType.bypass, replica_groups=RG,
                        ins=[ag_in.opt()], outs=[T_in.opt()])
                if l == 0 and LVL <= ORDER.index("layer0"):
                    bail()
                    STOP = True
                    break

            # ---- readout: all-reduce partial sums, mean, classify ----
            if not STOP:
                nc.sync.dma_start(ar_in[:, :], roacc_t[:])
                nc.gpsimd.collective_compute(
                    "AllReduce", mybir.AluOpType.add, replica_groups=RG,
                    ins=[ar_in.opt()], outs=[ar_out.opt()])
                nc.sync.dma_start(ro2_t[:], ar_out[:, :])
                nc.vector.tensor_scalar_max(cnt_t[:], ro2_t[:, F : F + 1], 1.0)
                nc.vector.reciprocal(rcp_t[:], cnt_t[:])
                nc.vector.tensor_tensor(hg_t[:], ro2_t[:, 0:F],
                                        rcp_t[:].broadcast_to([128, F]),
                                        mybir.AluOpType.mult)
                ptf = ptr.tile([128, 128], FP, name="ptf", tag="tr")
                nc.tensor.transpose(ptf[:], hg_t[:], ident_f[:])
                nc.vector.tensor_copy(hgT_t[:], ptf[:])
                plog = pro.tile([128, F + 1], FP, name="plog", tag="ro")
                nc.tensor.matmul(plog[:, 0:CLASSES], hgT_t[:],
                                 misc_t[:, MC_WC : MC_WC + CLASSES],
                                 start=True, stop=True)
                nc.vector.tensor_tensor(logit_t[:], plog[:, 0:CLASSES],
                                        misc_t[:, MC_BC : MC_BC + CLASSES],
                                        mybir.AluOpType.add)
                nc.sync.dma_start(out_d[:, :], logit_t[:])

    nc.finalize()
    return nc


def _make_in_maps(x, graph_ids, Ws, bs, Wc, bc, idx_c, slot_cols):
    b_cols = np.stack(bs, axis=1).astype(np.float32)            # [128, 3]
    bc_rep = np.tile(np.asarray(bc, np.float32)[None, :], (128, 1))
    # permute W0's input rows to match the int6 unpack column order
    # (device column j*32+k holds original feature 4k+j), same perm in
    # each of the 3 hop blocks; W1/W2 consume unpermuted h -> untouched.
    Q = F // 4
    perm = np.array([4 * k + j for j in range(4) for k in range(Q)])
    W0p = np.asarray(Ws[0], np.float32).reshape(HOPS + 1, F, F)[:, perm, :]
    Ws = [W0p.reshape((HOPS + 1) * F, F)] + [np.asarray(w) for w in Ws[1:]]
    w_bf = np.concatenate([np.asarray(w, np.float32) for w in Ws], axis=0).astype(NPBF)
    wc_f = np.asarray(Wc, np.float32)
    # per-node int6 offset-binary quantization of x, 4 values per 3 bytes
    xs_full = np.ones(N, np.float32)
    amax = np.abs(x).max(axis=1)
    nz = amax > 0
    xs_full[nz] = amax[nz] / 31.0
    q = (np.clip(np.round(x / xs_full[:, None]), -31, 31) + 32).astype(np.int32)
    qq = q.reshape(N, Q, 4)
    bits = qq[:, :, 0] | (qq[:, :, 1] << 6) | (qq[:, :, 2] << 12) | (qq[:, :, 3] << 18)
    xbytes = np.stack([bits & 255, (bits >> 8) & 255, (bits >> 16) & 255],
                      axis=1).astype(np.uint8)                  # [N, 3, Q]
    # weights packed slot-major: [128, 9*256] bytes
    w_pack = np.ascontiguousarray(
        w_bf.reshape(3 * (HOPS + 1), 128, F).transpose(1, 0, 2)
    ).view(np.int8).reshape(128, -1)
    in_maps = []
    for c in range(NCORES):
        # pad rows decode to q=32 everywhere -> (32-32)*scale = 0
        pad_bits = 32 | (32 << 6) | (32 << 12) | (32 << 18)
        x_loc = np.empty((NPAD, 3, Q), np.uint8)
        x_loc[:, 0] = pad_bits & 255
        x_loc[:, 1] = (pad_bits >> 8) & 255
        x_loc[:, 2] = (pad_bits >> 16) & 255
        x_loc[:PER] = xbytes[c * PER : (c + 1) * PER]
        x_pack = np.ascontiguousarray(
            x_loc.reshape(GRP, 128, XB).transpose(1, 0, 2)
        ).reshape(128, GRP * XB).view(np.int8)
        xs = np.ones(NPAD, np.float32)
        xs[:PER] = xs_full[c * PER : (c + 1) * PER]
        gsl = np.full(NPAD, 255.0, np.float32)
        gsl[:PER] = graph_ids[c * PER : (c + 1) * PER].astype(np.float32)
        misc = np.concatenate([
            b_cols, wc_f, bc_rep,
            np.ascontiguousarray(xs.reshape(GRP, 128).T),
            np.ascontiguousarray((-32.0 * xs).reshape(GRP, 128).T),
            np.ascontiguousarray(gsl.reshape(GRP, 128).T),
        ], axis=1).astype(np.float32)
        W16 = idx_c.shape[2]
        idx_pack = np.ascontiguousarray(
            idx_c[c].reshape(16, 8, W16 // 8).transpose(1, 0, 2)
        ).reshape(128, W16 // 8).view(np.int8)
        parts = [x_pack, slot_cols[c].view(np.int8), w_pack,
                 np.ascontiguousarray(misc).view(np.int8), idx_pack]
        X_OFF, SLOT_OFF, W_OFF, MISC_OFF, IDX_OFF, PCOLS = _pack_offsets(
            slot_cols.shape[2])
        pack = np.zeros((128, PCOLS), np.int8)
        for p, o in zip(parts, (X_OFF, SLOT_OFF, W_OFF, MISC_OFF, IDX_OFF)):
            pack[:, o : o + p.shape[1]] = p
        in_maps.append(dict(pack=pack))
    return in_maps


XB = F // 4 * 3              # packed int6 bytes per node (4 values / 3 bytes)


def _pack_offsets(NCH):
    MCOLS = 3 + CLASSES + CLASSES + GRP + GRP + GRP
    W16 = NCH * 128 // 16
    SLOT_OFF = GRP * XB
    W_OFF = -(-(SLOT_OFF + NCH) // 4) * 4
    MISC_OFF = -(-(W_OFF + (HLAYERS + 1) * (HOPS + 1) * F * 2) // 4) * 4
    IDX_OFF = MISC_OFF + MCOLS * 4
    PCOLS = IDX_OFF + (W16 // 8) * 2
    return 0, SLOT_OFF, W_OFF, MISC_OFF, IDX_OFF, PCOLS


def kernel(x, src, dst, graph_ids, W0, b0, W1, b1, W2, b2, Wc, bc, **_):
    x = np.asarray(x, np.float32)
    graph_ids = np.asarray(graph_ids, np.int64)
    idx_c, slot_cols, CAu, CBu = _prep_edges(src, dst)
    nc = _build_program(CAu, CBu)
    in_maps = _make_in_maps(
        x, graph_ids,
        [np.asarray(W0), np.asarray(W1), np.asarray(W2)],
        [np.asarray(b0, np.float32), np.asarray(b1, np.float32),
         np.asarray(b2, np.float32)],
        Wc, bc, idx_c, slot_cols)
    res = bass_utils.run_bass_kernel_spmd(nc, in_maps, core_ids=list(range(NCORES)))
    return np.asarray(res.results[0]["out"], np.float32)
